# revision 53
# baseline (speedup 1.0000x reference)
"""GPT-style dense transformer on 8 Trainium2 NeuronCores.

Sharding: token-parallel. Core c owns positions t = 8*i + c of BOTH batches
(256 positions per batch -> 512 tokens per core). All per-token work (LN,
qkv, out_proj, ff, lm_head) is local; attention needs all keys, so K^T and V
are AllGathered across the 8 cores once per layer per BATCH. The strided
assignment makes every core's causal structure identical (block-lower-
triangular over local indices, with a per-source-core diagonal mask), so one
SPMD program serves all cores.

Perf structure (v5, batch-pipelined; ~2.9ms v4 -> ~2.6ms):
- The layer is software-pipelined BY BATCH: per layer the schedule is
  attn(b0) | out/LN2/FF(b0) | LN1'+KV'(b0)+AG'(b0) | attn(b1) | ... |
  LN1'+KV'(b1)+AG'(b1) | Q'. Each 3.2MB AllGather is triggered ~100us of
  compute ahead of its consumer, so the collectives fly under compute.
  (8-rank RDH with Shared output is the fast path on this stack.)
- Attention scores per HEAD PAIR: heads (2m, 2m+1) occupy PE rows 0:64 /
  64:128 (K=64 each); adjacent issue with disjoint row groups runs both
  concurrently. The pair's scores share one 2-bank PSUM slot (hh0 bank0,
  hh1 bank1) -> ONE strided exp [128, 2, 384] per (core, pair) on ACT.
- IMPORTANT PSUM RULE (hardware): concurrent accumulation groups must live
  in DIFFERENT 2KB PSUM banks. Two interleaved start/stop groups in one bank
  corrupt each other (all-NaN + intermittent hangs). Hence y accumulators
  are one PSUM tile per head.
- Causal diag masks: multiplicative DVE ops on the exp'd scores (the es
  tiles), 2 x [128, 2, 128] per (core, pair); masks pre-tiled host-side.
- V is staged through the collective already padded with the softmax-ones
  column (65 cols/head); y and the softmax denominator accumulate in one
  PSUM pass. Normalize: ONE reciprocal_approx_fast per pair (both heads),
  broadcast via K=1 matmul into psA, final multiply on the idle GPSIMD.
- LN: stats via K=1 matmuls (1/D folded), rsqrt path uses
  reciprocal_approx_fast (5x faster than DVE reciprocal).
- Weights are STREAMED as pre-tiled [128, 768] slabs (one contiguous DMA
  each) instead of held resident: frees ~21KB/partition of SBUF.
- lm_head: embedding pre-tiled per 1024-vocab chunk [p, vc, d, v] so each
  chunk load is one contiguous-per-partition DMA; logits written in a
  core-tiled layout and untangled on the host. PE-bound at ~99% occupancy.
"""

import sys

for _p in ("/opt/trn_rl_repo",):
    if _p not in sys.path:
        sys.path.insert(0, _p)

import numpy as np
import ml_dtypes

import concourse.bass as bass
import concourse.bacc as bacc
import concourse.mybir as mybir
import concourse.tile as tile
from concourse.masks import make_identity

BF16 = mybir.dt.bfloat16
FP8 = mybir.dt.float8e4
F32 = mybir.dt.float32
I32 = mybir.dt.int32
AF = mybir.ActivationFunctionType
ALU = mybir.AluOpType

NCORES = 8
H = 12          # heads
HD = 64         # head dim
D = 768
D3 = 3 * D      # 2304
DF = 4 * D      # 3072
KD = D // 128   # 6 d-tiles
EPS = 1e-5

bf16 = ml_dtypes.bfloat16
WARM_AG = False
PIPELINE = True


def build_nc(nb, L, V, stop_at=None):
    """Build the SPMD Bass module. nb = 128-token tiles per (core, batch).
    Full size: nb=2 -> 512 tokens/core, T = 8*128*nb = 2048."""
    assert nb == 2, "v4 kernel is specialized to nb=2 (512 tokens/core)"
    NT = 2 * nb          # token tiles per core (4)
    PT = NT * 128        # tokens per core (512)
    TB = nb * 128        # tokens per batch per core (256)
    VW = 65              # V cols per head incl. ones column
    VH12 = H * VW        # V cols per key-token tile (all 12 heads, 780)
    KB = 6 * TB          # K^T staging cols per batch (6 head-pairs x 256)
    VB = 2 * VH12        # V staging cols per batch (2 key tiles x 780)
    TOT2 = 128 * (KB + VB)       # kv elems per rank per batch AllGather

    nc = bacc.Bacc("TRN2", target_bir_lowering=False, num_devices=NCORES)

    # ---- I/O ----
    NV2 = (V + 1023) // 1024          # 1024-wide vocab chunks for lm_head
    idxs = nc.dram_tensor("idxs", [128, NT], I32, kind="ExternalInput")
    posT = nc.dram_tensor("posT", [D, PT], BF16, kind="ExternalInput")
    # per-src-core causal mask for the two diagonal 128x128 blocks, replicated
    # for the two heads of a pair (h=2, q=128): two 3-D DVE multiplies per
    # (core, pair)
    masks2 = nc.dram_tensor("masks2", [128, NCORES * 256], BF16, kind="ExternalInput")
    toke = nc.dram_tensor("toke", [V, D], F32, kind="ExternalInput")
    # lm_head embedding pre-tiled per 1024-vocab chunk: [p, vc, d, v] layout so
    # each chunk load is one contiguous-per-partition DMA (128 descriptors)
    embC = nc.dram_tensor("embC", [128, NV2 * KD * 1024], BF16, kind="ExternalInput")
    # Q/K weights pre-tiled per output slab: [p, ot, d*128+c] so each slab is
    # one contiguous-per-partition DMA; V weights pre-tiled per d-tile.
    wqkC = [nc.dram_tensor(f"wqkC{l}", [128, 12 * D], BF16, kind="ExternalInput") for l in range(L)]
    wvC = [nc.dram_tensor(f"wvC{l}", [128, KD * D], BF16, kind="ExternalInput") for l in range(L)]
    bqkv = [nc.dram_tensor(f"bqkv{l}", [128, 12], F32, kind="ExternalInput") for l in range(L)]
    bqv = [nc.dram_tensor(f"bqv{l}", [1, D], F32, kind="ExternalInput") for l in range(L)]
    wout = [nc.dram_tensor(f"wout{l}", [D, D], BF16, kind="ExternalInput") for l in range(L)]
    w1C = [nc.dram_tensor(f"w1C{l}", [128, 24 * D], BF16, kind="ExternalInput") for l in range(L)]
    b1 = [nc.dram_tensor(f"b1_{l}", [128, 24], F32, kind="ExternalInput") for l in range(L)]
    w2 = [nc.dram_tensor(f"w2_{l}", [DF, D], BF16, kind="ExternalInput") for l in range(L)]
    # logits in core-tiled layout [p, vc, t, v]; host untangles (cheap)
    logits_t = nc.dram_tensor("logits_t", [128, NV2 * NT * 1024], BF16,
                              kind="ExternalOutput")

    from contextlib import ExitStack
    with tile.TileContext(nc) as tc, ExitStack() as ctx:
        def pool(**kw):
            return ctx.enter_context(tc.tile_pool(**kw))
        # ---- pools ----
        const = pool(name="const", bufs=1)
        resid = pool(name="resid", bufs=1)
        acts = pool(name="acts", bufs=1)
        kvres = pool(name="kvres", bufs=1)
        wpool = pool(name="wpool", bufs=1)
        wopool = pool(name="wopool", bufs=1)
        biasp = pool(name="biasp", bufs=2)
        rot = pool(name="rot", bufs=2)
        esp = pool(name="esp", bufs=6)
        gp = pool(name="gp", bufs=2)
        w2p = pool(name="w2p", bufs=2)
        embp = pool(name="embp", bufs=2)
        logp = pool(name="logp", bufs=2)
        rowp = pool(name="rowp", bufs=3)
        psA = pool(name="psA", bufs=3, space="PSUM")   # [128,1024] 2-bank slots
        psY = pool(name="psY", bufs=2, space="PSUM")   # [128,512] 1-bank slots
        dram = pool(name="dram", bufs=2, space="DRAM")

        def psa():
            return psA.tile([128, 1024], F32, name="sa", tag="s2")

        def psy():
            return psY.tile([128, PT], F32, name="sy", tag="y")

        # ---- constants ----
        ident = const.tile([128, 128], F32, name="ident", tag="ident")
        make_identity(nc, ident)
        ones_col = const.tile([128, 1], BF16, name="ones_col", tag="ones_col")
        nc.gpsimd.memset(ones_col[:, :], 1.0)
        invd_col = const.tile([128, 1], BF16, name="invd_col", tag="invd_col")
        nc.gpsimd.memset(invd_col[:, :], 1.0 / D)
        ones_row = const.tile([1, 128], F32, name="ones_row", tag="ones_row")
        nc.gpsimd.memset(ones_row[:, :], 1.0)
        eps_t = const.tile([1, 1], F32, name="eps_t", tag="eps_t")
        nc.gpsimd.memset(eps_t[:, :], EPS)
        zero_col = const.tile([128, 1], F32, name="zero_col", tag="zero_col")
        nc.gpsimd.memset(zero_col[:, :], 0.0)
        m2_sb = const.tile([128, NCORES * 256], BF16, name="m2_sb", tag="m2_sb")
        nc.sync.dma_start(out=m2_sb[:, :], in_=masks2[:, :])
        idx_sb = const.tile([128, NT], I32, name="idx_sb", tag="idx_sb")
        nc.sync.dma_start(out=idx_sb[:, :], in_=idxs[:, :])

        # tiny warm-up AllGather: wakes the collectives firmware during the
        # embedding phase so layer 0's first real AllGather doesn't pay the
        # ~100us first-collective latency observed in traces.
        if WARM_AG:
            warm_in = dram.tile([1, 256], BF16, name="warm_in", tag="warm_in")
            warm_out = dram.tile([1, NCORES * 256], BF16, name="warm_out",
                                 tag="warm_out", addr_space="Shared")
            nc.sync.dma_start(out=warm_in[:, :], in_=posT[0:1, 0:256])
            nc.gpsimd.collective_compute(
                "AllGather", ALU.bypass, replica_groups=[list(range(NCORES))],
                ins=[warm_in[:, :].opt()], outs=[warm_out[:, :].opt()])
            warm_sb = const.tile([1, 256], BF16, name="warm_sb", tag="warm_sb")
            nc.sync.dma_start(out=warm_sb[:, :], in_=warm_out[:, 0:256])

        # ---- persistent per-layer state ----
        xT = [resid.tile([128, PT], F32, name=f"xt{d}", tag=f"xt{d}") for d in range(KD)]
        hT = [acts.tile([128, PT], BF16, name=f"ht{d}", tag=f"ht{d}") for d in range(KD)]
        qT = [acts.tile([128, PT], BF16, name=f"qt{d}", tag=f"qt{d}") for d in range(KD)]
        yT = [acts.tile([128, PT], BF16, name=f"yt{d}", tag=f"yt{d}") for d in range(KD)]
        # gathered K^T + V per source core (ONE batch at a time), fused in ONE
        # tile so each (core, batch) load is a single contiguous-per-partition
        # DMA. ktg view: [128 pair-rows, 6 pairs, 256 tok]; vgf view:
        # [128 tok, 2 key tiles, 12 heads, 65].
        kvg = [kvres.tile([128, KB + VB], BF16, name=f"kvg{c}",
                          tag=f"kvg{c}") for c in range(NCORES)]
        ktg = [t[:, 0:KB] for t in kvg]
        vgf = [t[:, KB:KB + VB] for t in kvg]

        w768 = [wopool.tile([128, D], BF16, name=f"w7{i}", tag=f"w7{i}") for i in range(KD)]

        def wslab(src, ot):
            """Stream one [128, 6*128] weight slab (all 6 d-tiles of output
            slab ot) from a pre-tiled DRAM layout; one contiguous DMA."""
            t = wpool.tile([128, D], BF16, name="wsl", tag="wsl", bufs=4)
            nc.sync.dma_start(out=t[:, :], in_=src[:, ot * D:(ot + 1) * D])
            return t

        def ln_b(dst_bf16, _b):
            """dst[d][:, b*TB:(b+1)*TB] <- layernorm of batch-b cols of xT."""
            b = _b
            bc = slice(b * TB, (b + 1) * TB)
            s12 = psa()  # bank0: sum, bank1: sum of squares
            s1 = s12[0:1, 0:TB]
            s2 = s12[0:1, 512:512 + TB]
            for d in range(KD):
                xb = rot.tile([128, TB], BF16, name="xb", tag="xbb")
                nc.vector.tensor_copy(out=xb[:, :], in_=xT[d][:, bc])
                sq = rot.tile([128, TB], BF16, name="sq", tag="sqb")
                nc.vector.tensor_mul(out=sq[:, :], in0=xb[:, :], in1=xb[:, :])
                nc.tensor.matmul(out=s1, lhsT=invd_col[:, :], rhs=xb[:, :],
                                 start=(d == 0), stop=(d == KD - 1))
                nc.tensor.matmul(out=s2, lhsT=invd_col[:, :], rhs=sq[:, :],
                                 start=(d == 0), stop=(d == KD - 1))
            # s1 = E[x], s2 = E[x^2] directly (1/D folded into the matmul
            # constant); square on ACT to avoid a same-bank double PSUM read
            msq = rowp.tile([1, TB], F32, name="msq", tag="rowb")
            nc.scalar.activation(out=msq[:, :], in_=s1, func=AF.Square,
                                 bias=zero_col[0:1, :])
            vrow = rowp.tile([1, TB], F32, name="vrow", tag="rowb")
            nc.vector.tensor_tensor(out=vrow[:, :], in0=s2, in1=msq[:, :],
                                    op=ALU.subtract)
            srow = rowp.tile([1, TB], F32, name="srow", tag="rowb")
            nc.scalar.activation(out=srow[:, :], in_=vrow[:, :], func=AF.Sqrt,
                                 bias=eps_t[:, :])
            rrow = rowp.tile([1, TB], F32, name="rrow", tag="rowb")
            nc.vector.reciprocal_approx_fast(out=rrow[:, :], in_=srow[:, :])
            mr = rowp.tile([1, TB], F32, name="mr", tag="rowb")
            nc.vector.tensor_mul(out=mr[:, :], in0=s1, in1=rrow[:, :])
            # broadcast [1, TB] rows across 128 partitions via K=1 matmul
            bcpair = psa()
            bc_r = bcpair[:, 0:TB]
            bc_mr = bcpair[:, 512:512 + TB]
            nc.tensor.matmul(out=bc_r, lhsT=ones_row[:, :], rhs=rrow[:, :],
                             start=True, stop=True)
            nc.tensor.matmul(out=bc_mr, lhsT=ones_row[:, :], rhs=mr[:, :],
                             start=True, stop=True)
            for d in range(KD):
                t32 = rot.tile([128, TB], BF16, name="t32", tag="t32b")
                nc.vector.tensor_mul(out=t32[:, :], in0=xT[d][:, bc], in1=bc_r)
                nc.vector.tensor_sub(out=dst_bf16[d][:, bc], in0=t32[:, :], in1=bc_mr)

        # ---- per-layer phase helpers (batch-pipelined schedule) ----
        def load_kv_weights(lx):
            """Biases + V weights for layer lx's K/V projections, plus the
            V-bias broadcast to all 128 partitions."""
            bq2 = biasp.tile([128, 12], F32, name="bq", tag="bq")
            nc.sync.dma_start(out=bq2[:, :], in_=bqkv[lx][:, :])
            bv2 = biasp.tile([1, D], F32, name="bv", tag="bv")
            nc.sync.dma_start(out=bv2[:, :], in_=bqv[lx][:, :])
            wv2 = [wpool.tile([128, D], BF16, name=f"wv{d}", tag=f"wv{d}")
                   for d in range(KD)]
            for d in range(KD):
                nc.sync.dma_start(out=wv2[d][:, :], in_=wvC[lx][:, d * D:(d + 1) * D])
            bvb = rot.tile([128, D], F32, name="bvb", tag="bvb", bufs=2)
            for vh in range(2):
                bcv = psy()
                nc.tensor.matmul(out=bcv[:, 0:384], lhsT=ones_row[:, :],
                                 rhs=bv2[:, vh * 384:(vh + 1) * 384],
                                 start=True, stop=True)
                nc.vector.tensor_copy(out=bvb[:, vh * 384:(vh + 1) * 384],
                                      in_=bcv[:, 0:384])
            return bq2, wv2, bvb

        def kv_phase(lx, b, bq2, wv2, bvb):
            """LN1(lx, b) -> K^T/V projections for batch b (all 12 heads) ->
            stage to DRAM -> trigger the batch-b AllGather."""
            bc = slice(b * TB, (b + 1) * TB)
            ln_b(hT, b)
            kbig = rot.tile([128, KB], BF16, name="kbig", tag=f"kbig{b}", bufs=1)
            for ot in range(6, 12):
                wsl = wslab(wqkC[lx], ot)
                ps = psa()
                pv = ps[:, 0:TB]
                for d in range(KD):
                    nc.tensor.matmul(out=pv, lhsT=wsl[:, d * 128:(d + 1) * 128],
                                     rhs=hT[d][:, bc], start=(d == 0),
                                     stop=(d == KD - 1))
                nc.scalar.activation(out=kbig[:, (ot - 6) * TB:(ot - 5) * TB],
                                     in_=pv, func=AF.Identity, bias=bq2[:, ot:ot + 1])
            kv_in = dram.tile([1, TOT2], BF16, name="kv_in", tag=f"kv_in{b}")
            kv_out = dram.tile([1, NCORES * TOT2], BF16, name="kv_out",
                               tag=f"kv_out{b}", addr_space="Shared")
            kvf = kv_in[:, :].rearrange("o n -> (o n)")
            nc.sync.dma_start(
                out=kvf[0:TOT2].rearrange("(p x) -> p x", p=128)[:, 0:KB],
                in_=kbig[:, :])
            vbig = rot.tile([128, VB], BF16, name="vbig", tag=f"vbig{b}", bufs=1)
            vbig4 = vbig[:, :].rearrange("p (t h e) -> p t h e", t=2, h=H)
            nc.gpsimd.memset(vbig4[:, :, :, 64:65], 1.0)
            for tt in range(2):
                ps = psa()
                for d in range(KD):
                    for hv in range(2):  # out must stay within one PSUM bank
                        nc.tensor.matmul(
                            out=ps[:, hv * 512:hv * 512 + 512 - hv * 256],
                            lhsT=hT[d][:, b * TB + tt * 128:b * TB + (tt + 1) * 128],
                            rhs=wv2[d][:, hv * 512:hv * 512 + 512 - hv * 256],
                            start=(d == 0), stop=(d == KD - 1))
                nc.vector.tensor_tensor(
                    out=vbig4[:, tt, :, 0:64],
                    in0=ps[:, 0:D].rearrange("p (h e) -> p h e", e=64),
                    in1=bvb[:, :].rearrange("p (h e) -> p h e", e=64),
                    op=ALU.add)
            nc.sync.dma_start(
                out=kvf[0:TOT2].rearrange("(p x) -> p x", p=128)[:, KB:KB + VB],
                in_=vbig[:, :])
            nc.gpsimd.collective_compute(
                "AllGather", ALU.bypass, replica_groups=[list(range(NCORES))],
                ins=[kv_in[:, :].opt()], outs=[kv_out[:, :].opt()])
            return kv_out

        def q_phase(lx, bq2):
            for ot in range(6):
                wsl = wslab(wqkC[lx], ot)
                ps = psa()
                pv = ps[:, 0:PT]
                for d in range(KD):
                    nc.tensor.matmul(out=pv, lhsT=wsl[:, d * 128:(d + 1) * 128],
                                     rhs=hT[d][:, :], start=(d == 0),
                                     stop=(d == KD - 1))
                nc.scalar.activation(out=qT[ot][:, :], in_=pv,
                                     func=AF.Identity, bias=bq2[:, ot:ot + 1])

        def load_kvg(kvo_h):
            kvof = kvo_h[:, :].rearrange("o n -> (o n)")
            for c in range(NCORES):
                nc.sync.dma_start(
                    out=kvg[c][:, :],
                    in_=kvof[c * TOT2:(c + 1) * TOT2].rearrange("(p x) -> p x", p=128))

        def attn_b(b):
            """Attention for batch b over all 6 head pairs and 8 source cores.
            Pair scores share one 2-bank PSUM slot (hh0 bank0, hh1 bank1);
            causal diag masks applied with one strided DVE multiply per
            (core, pair); softmax normalize batched across the pair."""
            bc = slice(b * TB, (b + 1) * TB)
            for pr in range(6):
                qtile = qT[pr]
                # one accumulator tile PER HEAD: concurrent accumulation
                # groups must not share a PSUM bank
                y_ps = [psy(), psy()]
                for c in range(NCORES):
                    S = psa()
                    Sv = [S[:, 0:512], S[:, 512:1024]]
                    for kt in range(2):
                        for hh in range(2):
                            kp = hh * 64
                            nc.tensor.matmul(
                                out=Sv[hh][:, kt * 256:kt * 256 + 256 - kt * 128],
                                lhsT=ktg[c][kp:kp + 64,
                                            pr * TB + kt * 128:pr * TB + (kt + 1) * 128],
                                rhs=qtile[kp:kp + 64, b * TB + kt * 128:(b + 1) * TB],
                                start=(kt == 0), stop=(kt == 1))
                    es = esp.tile([128, 2 * 384], BF16, name="es", tag="es")
                    nc.scalar.activation(
                        out=es[:, :].rearrange("p (h q) -> p h q", h=2),
                        in_=S[:, :].rearrange("p (h q) -> p h q", h=2)[:, :, 0:384],
                        func=AF.Exp, bias=zero_col[:, :], scale=0.125)
                    es4 = es[:, :].rearrange("p (h a q) -> p h a q", h=2, q=128)
                    m2v = m2_sb[:, c * 256:(c + 1) * 256].rearrange(
                        "p (h q) -> p h q", h=2)
                    for a in (0, 2):  # the two diagonal 128x128 blocks
                        nc.vector.tensor_tensor(
                            out=es4[:, :, a, :], in0=es4[:, :, a, :],
                            in1=m2v, op=ALU.mult)
                    es2 = es[:, :].rearrange("p (h q) -> p h q", h=2)
                    for hh in range(2):
                        hx = 2 * pr + hh
                        nc.tensor.matmul(
                            out=y_ps[hh][0:VW, 0:256],
                            lhsT=vgf[c][:, hx * VW:(hx + 1) * VW],
                            rhs=es2[:, hh, 0:256], start=(c == 0), stop=False)
                        nc.tensor.matmul(
                            out=y_ps[hh][0:VW, 128:256],
                            lhsT=vgf[c][:, VH12 + hx * VW:VH12 + (hx + 1) * VW],
                            rhs=es2[:, hh, 256:384],
                            start=False, stop=(c == NCORES - 1))
                # evacuate fast; normalize (one reciprocal per PAIR) off-path
                y_sb = rot.tile([65, PT], BF16, name="y_sb", tag="y_sb", bufs=3)
                zden = rowp.tile([1, PT], F32, name="zden", tag="row")
                for hh in range(2):
                    nc.vector.tensor_copy(out=y_sb[0:65, hh * 256:(hh + 1) * 256],
                                          in_=y_ps[hh][0:65, 0:256])
                    nc.vector.tensor_copy(out=zden[:, hh * 256:(hh + 1) * 256],
                                          in_=y_ps[hh][64:65, 0:256])
                zrec = rowp.tile([1, PT], F32, name="zrec", tag="row")
                nc.vector.reciprocal_approx_fast(out=zrec[:, :], in_=zden[:, :])
                bcp = psa()  # keep psY free for the next pair's accumulators
                bcb = bcp[:, 0:512]
                nc.tensor.matmul(out=bcb[0:64, 0:PT], lhsT=ones_row[:, 0:64],
                                 rhs=zrec[:, :], start=True, stop=True)
                for hh in range(2):
                    hx = 2 * pr + hh
                    nc.vector.tensor_tensor(
                        out=yT[hx // 2][(hx % 2) * 64:(hx % 2) * 64 + 64, bc],
                        in0=y_sb[0:64, hh * 256:(hh + 1) * 256],
                        in1=bcb[0:64, hh * 256:(hh + 1) * 256], op=ALU.mult)

        def out_proj_b(b):
            bc = slice(b * TB, (b + 1) * TB)
            oslot = [psa() for _ in range(3)]
            oacc = [oslot[o // 2][:, (o % 2) * 512:(o % 2) * 512 + TB]
                    for o in range(KD)]
            for k in range(KD):
                for o in range(KD):
                    nc.tensor.matmul(out=oacc[o],
                                     lhsT=w768[k][:, o * 128:(o + 1) * 128],
                                     rhs=yT[k][:, bc], start=(k == 0),
                                     stop=(k == KD - 1))
            for o in range(KD):
                nc.vector.tensor_add(out=xT[o][:, bc], in0=xT[o][:, bc],
                                     in1=oacc[o])

        def ff_b(lx, b, bft):
            bc = slice(b * TB, (b + 1) * TB)
            fslot = [psa() for _ in range(3)]
            facc = [fslot[o // 2][:, (o % 2) * 512:(o % 2) * 512 + TB]
                    for o in range(KD)]
            for ot in range(24):
                wsl = wslab(w1C[lx], ot)
                ps = psy()
                for d in range(KD):
                    nc.tensor.matmul(out=ps[:, 0:TB],
                                     lhsT=wsl[:, d * 128:(d + 1) * 128],
                                     rhs=hT[d][:, bc], start=(d == 0),
                                     stop=(d == KD - 1))
                g = gp.tile([128, TB], BF16, name="g", tag="gb")
                nc.scalar.activation(out=g[:, :], in_=ps[:, 0:TB], func=AF.Gelu,
                                     bias=bft[:, ot:ot + 1], scale=1.0)
                w2s = w2p.tile([128, D], BF16, name="w2s", tag="w2s")
                nc.sync.dma_start(out=w2s[:, :], in_=w2[lx][ot * 128:(ot + 1) * 128, :])
                for o in range(KD):
                    nc.tensor.matmul(out=facc[o], lhsT=w2s[:, o * 128:(o + 1) * 128],
                                     rhs=g[:, :], start=(ot == 0), stop=(ot == 23))
            for o in range(KD):
                nc.vector.tensor_add(out=xT[o][:, bc], in0=xT[o][:, bc],
                                     in1=facc[o])

        # ================= embedding =================
        posv = rot.tile([128, KD * PT], BF16, name="posv", tag="posv", bufs=1)
        for d in range(KD):
            nc.sync.dma_start(out=posv[:, d * PT:(d + 1) * PT],
                              in_=posT[d * 128:(d + 1) * 128, :])

        def embed_tiles(tts):
            for tt in tts:
                xg = rot.tile([128, D], F32, name="xg", tag="xg", bufs=1)
                nc.gpsimd.indirect_dma_start(
                    out=xg[:, :], out_offset=None, in_=toke[:, :],
                    in_offset=bass.IndirectOffsetOnAxis(ap=idx_sb[:, tt:tt + 1], axis=0))
                for dp in range(3):  # d-pairs share a 2-bank slot
                    tp = psa()
                    for k in range(2):
                        d = 2 * dp + k
                        sub = tp[:, k * 512:k * 512 + 128]
                        nc.tensor.transpose(out=sub, in_=xg[:, d * 128:(d + 1) * 128],
                                            identity=ident[:, :])
                        nc.vector.tensor_tensor(
                            out=xT[d][:, tt * 128:(tt + 1) * 128], in0=sub,
                            in1=posv[:, d * PT + tt * 128:d * PT + (tt + 1) * 128],
                            op=ALU.add)

        # ================= pipelined prologue + layers =================
        # Steady state per layer: attn(b0) | out/LN2/FF(b0) | LN1'+KV'+AG'(b0)
        # | attn(b1) | out/LN2/FF(b1) | LN1'+KV'+AG'(b1) | Q'. Each AllGather
        # is triggered ~100us of compute before its consumer, so the
        # collectives fly entirely under compute.
        kvo = [None, None]
        if PIPELINE:
            bq2, wv2, bvb = load_kv_weights(0)
            embed_tiles([0, 1])
            kvo[0] = kv_phase(0, 0, bq2, wv2, bvb)
            embed_tiles([2, 3])
            kvo[1] = kv_phase(0, 1, bq2, wv2, bvb)
            q_phase(0, bq2)
            load_kvg(kvo[0])
        else:
            embed_tiles([0, 1, 2, 3])

        for l in range(L):
            for k in range(KD):
                nc.sync.dma_start(out=w768[k][:, :],
                                  in_=wout[l][k * 128:(k + 1) * 128, :])
            bft = biasp.tile([128, 24], F32, name="bft", tag="bft")
            nc.sync.dma_start(out=bft[:, :], in_=b1[l][:, :])
            if not PIPELINE:
                bq2, wv2, bvb = load_kv_weights(l)
                kvo[0] = kv_phase(l, 0, bq2, wv2, bvb)
                kvo[1] = kv_phase(l, 1, bq2, wv2, bvb)
                q_phase(l, bq2)
                load_kvg(kvo[0])
            elif l + 1 < L:
                bq2, wv2, bvb = load_kv_weights(l + 1)
            # prefetch the Exp activation table before attention
            texp = rowp.tile([1, PT], F32, name="texp", tag="row")
            nc.scalar.activation(out=texp[0:1, 0:1], in_=eps_t[:, :],
                                 func=AF.Exp, bias=eps_t[:, :])
            for b in range(2):
                attn_b(b)
                if b == 0:
                    load_kvg(kvo[1])
                out_proj_b(b)
                ln_b(hT, b)          # LN2
                ff_b(l, b, bft)
                if PIPELINE and l + 1 < L:
                    kvo[b] = kv_phase(l + 1, b, bq2, wv2, bvb)
            if PIPELINE and l + 1 < L:
                q_phase(l + 1, bq2)
                load_kvg(kvo[0])

        # ================= final LN + lm_head =================
        ln_b(hT, 0)
        ln_b(hT, 1)
        for vc in range(NV2):
            esl = embp.tile([128, KD * 1024], BF16, name="esl", tag="esl")
            nc.sync.dma_start(out=esl[:, :],
                              in_=embC[:, vc * KD * 1024:(vc + 1) * KD * 1024])
            esl3 = esl[:, :].rearrange("p (d v) -> p d v", v=1024)
            for tp in range(2):
                lsb = logp.tile([128, 2 * 1024], BF16, name="lsb", tag="lsb")
                lsb3 = lsb[:, :].rearrange("p (t v) -> p t v", v=1024)
                for k in range(2):
                    tt = 2 * tp + k
                    ps = psa()
                    for d in range(KD):
                        for hv in range(2):  # matmul out must stay in one bank
                            nc.tensor.matmul(out=ps[:, hv * 512:(hv + 1) * 512],
                                             lhsT=hT[d][:, tt * 128:(tt + 1) * 128],
                                             rhs=esl3[:, d, hv * 512:(hv + 1) * 512],
                                             start=(d == 0), stop=(d == KD - 1))
                    if tt % 2 == 0:
                        nc.vector.tensor_copy(out=lsb3[:, k, :], in_=ps[:, :])
                    else:
                        nc.scalar.activation(out=lsb3[:, k, :], in_=ps[:, :],
                                             func=AF.Identity, bias=zero_col[:, :])
                nc.sync.dma_start(
                    out=logits_t[:, vc * NT * 1024 + tp * 2048:
                                 vc * NT * 1024 + (tp + 1) * 2048],
                    in_=lsb[:, :])
    nc.finalize()
    return nc


# ------------------------------------------------------------------
# host side
# ------------------------------------------------------------------

def _prep_inputs(nb, L, V, idx, tok_emb, pos_emb, ln1_w, ln1_b, qkv_w, out_w,
                 ln2_w, ln2_b, ff1_w, ff2_w, lnf_w, lnf_b):
    NT = 2 * nb
    PT = NT * 128
    idx = np.asarray(idx).astype(np.int32)
    f = np.asarray

    V_ = tok_emb.shape[0]
    NV2 = (V_ + 1023) // 1024
    embW = (f(tok_emb, dtype=np.float32) * f(lnf_w, dtype=np.float32)[None, :]).T  # [D, V]
    embP = np.zeros((D, NV2 * 1024), np.float32)
    embP[:, :V_] = embW
    embC = embP.reshape(KD, 128, NV2, 1024).transpose(1, 2, 0, 3)
    shared = {
        "toke": f(tok_emb, dtype=np.float32),
        "embC": np.ascontiguousarray(embC.reshape(128, NV2 * KD * 1024)).astype(bf16),
    }
    for l in range(L):
        wq = f(qkv_w[l], dtype=np.float32) * f(ln1_w[l], dtype=np.float32)[:, None]
        bq_full = f(ln1_b[l], dtype=np.float32) @ f(qkv_w[l], dtype=np.float32)  # [3D]
        # Q/K slabs tiled [p, ot, d*128+c] (ot 0..5 = Q, 6..11 = K)
        wqk = wq[:, :2 * D].reshape(KD, 128, 12, 128).transpose(1, 2, 0, 3)
        shared[f"wqkC{l}"] = np.ascontiguousarray(wqk.reshape(128, 12 * D)).astype(bf16)
        # V weights tiled [p, d, hk*384+e]
        wv_ = wq[:, 2 * D:].reshape(KD, 128, D).transpose(1, 0, 2)
        shared[f"wvC{l}"] = np.ascontiguousarray(wv_.reshape(128, KD * D)).astype(bf16)
        shared[f"bqkv{l}"] = np.ascontiguousarray(bq_full[:12 * 128].reshape(12, 128).T).astype(np.float32)
        shared[f"bqv{l}"] = bq_full[2 * D:].reshape(1, D).astype(np.float32)
        shared[f"wout{l}"] = f(out_w[l], dtype=np.float32).astype(bf16)
        w1e = f(ff1_w[l], dtype=np.float32) * f(ln2_w[l], dtype=np.float32)[:, None]
        b1_full = f(ln2_b[l], dtype=np.float32) @ f(ff1_w[l], dtype=np.float32)  # [4D]
        w1t = w1e.reshape(KD, 128, 24, 128).transpose(1, 2, 0, 3)
        shared[f"w1C{l}"] = np.ascontiguousarray(w1t.reshape(128, 24 * D)).astype(bf16)
        shared[f"b1_{l}"] = np.ascontiguousarray(b1_full.reshape(24, 128).T).astype(np.float32)
        shared[f"w2_{l}"] = f(ff2_w[l], dtype=np.float32).astype(bf16)

    pos_f = f(pos_emb, dtype=np.float32)
    in_maps = []
    for c in range(NCORES):
        m = dict(shared)
        L_loc = np.arange(PT)
        b_loc = L_loc // (nb * 128)
        t_loc = 8 * (L_loc % (nb * 128)) + c
        idx_core = idx[b_loc, t_loc]  # [PT]
        m["idxs"] = np.ascontiguousarray(idx_core.reshape(NT, 128).T).astype(np.int32)
        m["posT"] = np.ascontiguousarray(pos_f[t_loc].T).astype(bf16)
        # diagonal-block causal masks per source core cp, replicated x2 for the
        # two heads of a pair: keep k <= q for cp <= c, k < q for cp > c.
        mk2 = np.zeros((128, NCORES * 256), dtype=np.float32)
        for cp in range(NCORES):
            mk = np.triu(np.ones((128, 128), np.float32), 0 if cp <= c else 1)
            mk2[:, cp * 256:(cp + 1) * 256] = np.tile(mk, (1, 2))
        m["masks2"] = mk2.astype(bf16)
        in_maps.append(m)
    return in_maps


_NC_CACHE = {}


def _get_nc(nb, L, V):
    key = (nb, L, V)
    if key not in _NC_CACHE:
        _NC_CACHE[key] = build_nc(nb, L, V)
    return _NC_CACHE[key]


def run_on_hw(nb, L, V, inputs, trace=False):
    from concourse import bass_utils
    nc = _get_nc(nb, L, V)
    in_maps = _prep_inputs(nb, L, V, **inputs)
    res = bass_utils.run_bass_kernel_spmd(nc, in_maps, core_ids=list(range(NCORES)),
                                          trace=trace)
    return res


def assemble(nb, L, V, results, lnf_b, tok_emb):
    T = 8 * nb * 128
    NT = 2 * nb
    NV2 = (V + 1023) // 1024
    out = np.empty((2, T, V), dtype=np.float32)
    for c in range(NCORES):
        lt = results[c]["logits_t"].astype(np.float32).reshape(128, NV2, NT, 1024)
        lg = lt.transpose(2, 0, 1, 3).reshape(NT * 128, NV2 * 1024)[:, :V]
        out[:, c::8, :] = lg.reshape(2, nb * 128, V)
    lnf_b = np.asarray(lnf_b, dtype=np.float32)
    if np.any(lnf_b):
        out += (lnf_b @ np.asarray(tok_emb, dtype=np.float32).T)[None, None, :]
    return out


def kernel(**inputs):
    nb, L, V = 2, 6, 32000
    res = run_on_hw(nb, L, V, inputs)
    return assemble(nb, L, V, res.results, inputs["lnf_b"], inputs["tok_emb"])



# revision 56
# speedup vs baseline: 1.0804x; 1.0804x over previous
"""GPT-style dense transformer on 8 Trainium2 NeuronCores.

Sharding: token-parallel. Core c owns positions t = 8*i + c of BOTH batches
(256 positions per batch -> 512 tokens per core). All per-token work (LN,
qkv, out_proj, ff, lm_head) is local; attention needs all keys, so K^T and V
are AllGathered across the 8 cores once per layer per BATCH. The strided
assignment makes every core's causal structure identical (block-lower-
triangular over local indices, with a per-source-core diagonal mask), so one
SPMD program serves all cores.

Perf structure (v5, batch-pipelined; ~2.9ms v4 -> ~2.6ms):
- The layer is software-pipelined BY BATCH: per layer the schedule is
  attn(b0) | out/LN2/FF(b0) | LN1'+KV'(b0)+AG'(b0) | attn(b1) | ... |
  LN1'+KV'(b1)+AG'(b1) | Q'. Each 3.2MB AllGather is triggered ~100us of
  compute ahead of its consumer, so the collectives fly under compute.
  (8-rank RDH with Shared output is the fast path on this stack.)
- Attention scores per HEAD PAIR: heads (2m, 2m+1) occupy PE rows 0:64 /
  64:128 (K=64 each); adjacent issue with disjoint row groups runs both
  concurrently. The pair's scores share one 2-bank PSUM slot (hh0 bank0,
  hh1 bank1) -> ONE strided exp [128, 2, 384] per (core, pair) on ACT.
- IMPORTANT PSUM RULE (hardware): concurrent accumulation groups must live
  in DIFFERENT 2KB PSUM banks. Two interleaved start/stop groups in one bank
  corrupt each other (all-NaN + intermittent hangs). Hence y accumulators
  are one PSUM tile per head.
- Causal diag masks: multiplicative DVE ops on the exp'd scores (the es
  tiles), 2 x [128, 2, 128] per (core, pair); masks pre-tiled host-side.
- V is staged through the collective already padded with the softmax-ones
  column (65 cols/head); y and the softmax denominator accumulate in one
  PSUM pass. Normalize: ONE reciprocal_approx_fast per pair (both heads),
  broadcast via K=1 matmul into psA, final multiply on the idle GPSIMD.
- LN: stats via K=1 matmuls (1/D folded), rsqrt path uses
  reciprocal_approx_fast (5x faster than DVE reciprocal).
- Weights are STREAMED as pre-tiled [128, 768] slabs (one contiguous DMA
  each) instead of held resident: frees ~21KB/partition of SBUF.
- lm_head: embedding pre-tiled per 1024-vocab chunk [p, vc, d, v] so each
  chunk load is one contiguous-per-partition DMA; logits written in a
  core-tiled layout and untangled on the host. PE-bound at ~99% occupancy.
"""

import sys

for _p in ("/opt/trn_rl_repo",):
    if _p not in sys.path:
        sys.path.insert(0, _p)

import numpy as np
import ml_dtypes

import concourse.bass as bass
import concourse.bacc as bacc
import concourse.mybir as mybir
import concourse.tile as tile
from concourse.masks import make_identity

BF16 = mybir.dt.bfloat16
FP8 = mybir.dt.float8e4
F32 = mybir.dt.float32
I32 = mybir.dt.int32
AF = mybir.ActivationFunctionType
ALU = mybir.AluOpType

NCORES = 8
H = 12          # heads
HD = 64         # head dim
D = 768
D3 = 3 * D      # 2304
DF = 4 * D      # 3072
KD = D // 128   # 6 d-tiles
EPS = 1e-5

bf16 = ml_dtypes.bfloat16
WARM_AG = False
PIPELINE = True


def build_nc(nb, L, V, stop_at=None):
    """Build the SPMD Bass module. nb = 128-token tiles per (core, batch).
    Full size: nb=2 -> 512 tokens/core, T = 8*128*nb = 2048."""
    assert nb == 2, "v4 kernel is specialized to nb=2 (512 tokens/core)"
    NT = 2 * nb          # token tiles per core (4)
    PT = NT * 128        # tokens per core (512)
    TB = nb * 128        # tokens per batch per core (256)
    VW = 65              # V cols per head incl. ones column
    VH12 = H * VW        # V cols per key-token tile (all 12 heads, 780)
    KB = 6 * TB          # K^T staging cols per batch (6 head-pairs x 256)
    VB = 2 * VH12        # V staging cols per batch (2 key tiles x 780)
    TOT2 = 128 * (KB + VB)       # kv elems per rank per batch AllGather

    nc = bacc.Bacc("TRN2", target_bir_lowering=False, num_devices=NCORES)

    # ---- I/O ----
    NV2 = (V + 1023) // 1024          # 1024-wide vocab chunks for lm_head
    idxs = nc.dram_tensor("idxs", [128, NT], I32, kind="ExternalInput")
    posT = nc.dram_tensor("posT", [D, PT], BF16, kind="ExternalInput")
    # per-src-core causal mask for the two diagonal 128x128 blocks, replicated
    # for the two heads of a pair (h=2, q=128): two 3-D DVE multiplies per
    # (core, pair)
    masks2 = nc.dram_tensor("masks2", [128, NCORES * 256], BF16, kind="ExternalInput")
    toke = nc.dram_tensor("toke", [V, D], F32, kind="ExternalInput")
    # lm_head embedding pre-tiled per 1024-vocab chunk: [p, vc, d, v] layout so
    # each chunk load is one contiguous-per-partition DMA (128 descriptors)
    embC = nc.dram_tensor("embC", [128, NV2 * KD * 1024], BF16, kind="ExternalInput")
    # Q/K weights pre-tiled per output slab: [p, ot, d*128+c] so each slab is
    # one contiguous-per-partition DMA; V weights pre-tiled per d-tile.
    wqkC = [nc.dram_tensor(f"wqkC{l}", [128, 12 * D], BF16, kind="ExternalInput") for l in range(L)]
    wvC = [nc.dram_tensor(f"wvC{l}", [128, KD * D], BF16, kind="ExternalInput") for l in range(L)]
    bqkv = [nc.dram_tensor(f"bqkv{l}", [128, 12], F32, kind="ExternalInput") for l in range(L)]
    bqv = [nc.dram_tensor(f"bqv{l}", [1, D], F32, kind="ExternalInput") for l in range(L)]
    wout = [nc.dram_tensor(f"wout{l}", [D, D], BF16, kind="ExternalInput") for l in range(L)]
    w1C = [nc.dram_tensor(f"w1C{l}", [128, 24 * D], BF16, kind="ExternalInput") for l in range(L)]
    b1 = [nc.dram_tensor(f"b1_{l}", [128, 24], F32, kind="ExternalInput") for l in range(L)]
    w2 = [nc.dram_tensor(f"w2_{l}", [DF, D], BF16, kind="ExternalInput") for l in range(L)]
    # logits in core-tiled layout [p, vc, t, v]; host untangles (cheap)
    logits_t = nc.dram_tensor("logits_t", [128, NV2 * NT * 1024], BF16,
                              kind="ExternalOutput")

    from contextlib import ExitStack
    with tile.TileContext(nc) as tc, ExitStack() as ctx:
        def pool(**kw):
            return ctx.enter_context(tc.tile_pool(**kw))
        # ---- pools ----
        const = pool(name="const", bufs=1)
        resid = pool(name="resid", bufs=1)
        acts = pool(name="acts", bufs=1)
        kvres = pool(name="kvres", bufs=1)
        wpool = pool(name="wpool", bufs=1)
        wopool = pool(name="wopool", bufs=1)
        biasp = pool(name="biasp", bufs=2)
        rot = pool(name="rot", bufs=2)
        esp = pool(name="esp", bufs=4)
        gp = pool(name="gp", bufs=2)
        w2p = pool(name="w2p", bufs=2)
        embp = pool(name="embp", bufs=2)
        logp = pool(name="logp", bufs=2)
        rowp = pool(name="rowp", bufs=3)
        psA = pool(name="psA", bufs=3, space="PSUM")   # [128,1024] 2-bank slots
        psY = pool(name="psY", bufs=2, space="PSUM")   # [128,512] 1-bank slots
        dram = pool(name="dram", bufs=2, space="DRAM")

        def psa():
            return psA.tile([128, 1024], F32, name="sa", tag="s2")

        def psy():
            return psY.tile([128, PT], F32, name="sy", tag="y")

        # ---- constants ----
        ident = const.tile([128, 128], F32, name="ident", tag="ident")
        make_identity(nc, ident)
        ones_col = const.tile([128, 1], BF16, name="ones_col", tag="ones_col")
        nc.gpsimd.memset(ones_col[:, :], 1.0)
        invd_col = const.tile([128, 1], BF16, name="invd_col", tag="invd_col")
        nc.gpsimd.memset(invd_col[:, :], 1.0 / D)
        ones_row = const.tile([1, 128], F32, name="ones_row", tag="ones_row")
        nc.gpsimd.memset(ones_row[:, :], 1.0)
        eps_t = const.tile([1, 1], F32, name="eps_t", tag="eps_t")
        nc.gpsimd.memset(eps_t[:, :], EPS)
        zero_col = const.tile([128, 1], F32, name="zero_col", tag="zero_col")
        nc.gpsimd.memset(zero_col[:, :], 0.0)
        m2_sb = const.tile([128, NCORES * 256], BF16, name="m2_sb", tag="m2_sb")
        nc.sync.dma_start(out=m2_sb[:, :], in_=masks2[:, :])
        idx_sb = const.tile([128, NT], I32, name="idx_sb", tag="idx_sb")
        nc.sync.dma_start(out=idx_sb[:, :], in_=idxs[:, :])

        # tiny warm-up AllGather: wakes the collectives firmware during the
        # embedding phase so layer 0's first real AllGather doesn't pay the
        # ~100us first-collective latency observed in traces.
        if WARM_AG:
            warm_in = dram.tile([1, 256], BF16, name="warm_in", tag="warm_in")
            warm_out = dram.tile([1, NCORES * 256], BF16, name="warm_out",
                                 tag="warm_out", addr_space="Shared")
            nc.sync.dma_start(out=warm_in[:, :], in_=posT[0:1, 0:256])
            nc.gpsimd.collective_compute(
                "AllGather", ALU.bypass, replica_groups=[list(range(NCORES))],
                ins=[warm_in[:, :].opt()], outs=[warm_out[:, :].opt()])
            warm_sb = const.tile([1, 256], BF16, name="warm_sb", tag="warm_sb")
            nc.sync.dma_start(out=warm_sb[:, :], in_=warm_out[:, 0:256])

        # ---- persistent per-layer state ----
        xT = [resid.tile([128, PT], F32, name=f"xt{d}", tag=f"xt{d}") for d in range(KD)]
        hT = [acts.tile([128, PT], BF16, name=f"ht{d}", tag=f"ht{d}") for d in range(KD)]
        qT = [acts.tile([128, PT], BF16, name=f"qt{d}", tag=f"qt{d}") for d in range(KD)]
        yT = [acts.tile([128, PT], BF16, name=f"yt{d}", tag=f"yt{d}") for d in range(KD)]
        # gathered K^T + V per source core (ONE batch at a time), fused in ONE
        # tile so each (core, batch) load is a single contiguous-per-partition
        # DMA. ktg view: [128 pair-rows, 6 pairs, 256 tok]; vgf view:
        # [128 tok, 2 key tiles, 12 heads, 65].
        kvg = [kvres.tile([128, KB + VB], BF16, name=f"kvg{c}",
                          tag=f"kvg{c}") for c in range(NCORES)]
        ktg = [t[:, 0:KB] for t in kvg]
        vgf = [t[:, KB:KB + VB] for t in kvg]

        w768 = [wopool.tile([128, D], BF16, name=f"w7{i}", tag=f"w7{i}") for i in range(KD)]

        def wslab(src, ot):
            """Stream one [128, 6*128] weight slab (all 6 d-tiles of output
            slab ot) from a pre-tiled DRAM layout; one contiguous DMA."""
            t = wpool.tile([128, D], BF16, name="wsl", tag="wsl", bufs=4)
            nc.sync.dma_start(out=t[:, :], in_=src[:, ot * D:(ot + 1) * D])
            return t

        def ln_b(dst_bf16, _b):
            """dst[d][:, b*TB:(b+1)*TB] <- layernorm of batch-b cols of xT."""
            b = _b
            bc = slice(b * TB, (b + 1) * TB)
            s12 = psa()  # bank0: sum, bank1: sum of squares
            s1 = s12[0:1, 0:TB]
            s2 = s12[0:1, 512:512 + TB]
            for d in range(KD):
                xb = rot.tile([128, TB], BF16, name="xb", tag="xbb")
                nc.vector.tensor_copy(out=xb[:, :], in_=xT[d][:, bc])
                sq = rot.tile([128, TB], BF16, name="sq", tag="sqb")
                nc.vector.tensor_mul(out=sq[:, :], in0=xb[:, :], in1=xb[:, :])
                nc.tensor.matmul(out=s1, lhsT=invd_col[:, :], rhs=xb[:, :],
                                 start=(d == 0), stop=(d == KD - 1))
                nc.tensor.matmul(out=s2, lhsT=invd_col[:, :], rhs=sq[:, :],
                                 start=(d == 0), stop=(d == KD - 1))
            # s1 = E[x], s2 = E[x^2] directly (1/D folded into the matmul
            # constant); square on ACT to avoid a same-bank double PSUM read
            msq = rowp.tile([1, TB], F32, name="msq", tag="rowb")
            nc.scalar.activation(out=msq[:, :], in_=s1, func=AF.Square,
                                 bias=zero_col[0:1, :])
            vrow = rowp.tile([1, TB], F32, name="vrow", tag="rowb")
            nc.vector.tensor_tensor(out=vrow[:, :], in0=s2, in1=msq[:, :],
                                    op=ALU.subtract)
            srow = rowp.tile([1, TB], F32, name="srow", tag="rowb")
            nc.scalar.activation(out=srow[:, :], in_=vrow[:, :], func=AF.Sqrt,
                                 bias=eps_t[:, :])
            rrow = rowp.tile([1, TB], F32, name="rrow", tag="rowb")
            nc.vector.reciprocal_approx_fast(out=rrow[:, :], in_=srow[:, :])
            mr = rowp.tile([1, TB], F32, name="mr", tag="rowb")
            nc.vector.tensor_mul(out=mr[:, :], in0=s1, in1=rrow[:, :])
            # broadcast [1, TB] rows across 128 partitions via K=1 matmul
            bcpair = psa()
            bc_r = bcpair[:, 0:TB]
            bc_mr = bcpair[:, 512:512 + TB]
            nc.tensor.matmul(out=bc_r, lhsT=ones_row[:, :], rhs=rrow[:, :],
                             start=True, stop=True)
            nc.tensor.matmul(out=bc_mr, lhsT=ones_row[:, :], rhs=mr[:, :],
                             start=True, stop=True)
            for d in range(KD):
                t32 = rot.tile([128, TB], BF16, name="t32", tag="t32b")
                nc.vector.tensor_mul(out=t32[:, :], in0=xT[d][:, bc], in1=bc_r)
                nc.vector.tensor_sub(out=dst_bf16[d][:, bc], in0=t32[:, :], in1=bc_mr)

        # ---- per-layer phase helpers (batch-pipelined schedule) ----
        def load_kv_weights(lx):
            """Biases + V weights for layer lx's K/V projections, plus the
            V-bias broadcast to all 128 partitions."""
            bq2 = biasp.tile([128, 12], F32, name="bq", tag="bq")
            nc.sync.dma_start(out=bq2[:, :], in_=bqkv[lx][:, :])
            bv2 = biasp.tile([1, D], F32, name="bv", tag="bv")
            nc.sync.dma_start(out=bv2[:, :], in_=bqv[lx][:, :])
            wv2 = [wpool.tile([128, D], BF16, name=f"wv{d}", tag=f"wv{d}")
                   for d in range(KD)]
            for d in range(KD):
                nc.sync.dma_start(out=wv2[d][:, :], in_=wvC[lx][:, d * D:(d + 1) * D])
            bvb = rot.tile([128, D], F32, name="bvb", tag="bvb", bufs=2)
            for vh in range(2):
                bcv = psy()
                nc.tensor.matmul(out=bcv[:, 0:384], lhsT=ones_row[:, :],
                                 rhs=bv2[:, vh * 384:(vh + 1) * 384],
                                 start=True, stop=True)
                nc.vector.tensor_copy(out=bvb[:, vh * 384:(vh + 1) * 384],
                                      in_=bcv[:, 0:384])
            return bq2, wv2, bvb

        def kv_phase(lx, b, bq2, wv2, bvb):
            """LN1(lx, b) -> K^T/V projections for batch b (all 12 heads) ->
            stage to DRAM -> trigger the batch-b AllGather."""
            bc = slice(b * TB, (b + 1) * TB)
            ln_b(hT, b)
            kbig = rot.tile([128, KB], BF16, name="kbig", tag=f"kbig{b}", bufs=1)
            for ot in range(6, 12):
                wsl = wslab(wqkC[lx], ot)
                ps = psa()
                pv = ps[:, 0:TB]
                for d in range(KD):
                    nc.tensor.matmul(out=pv, lhsT=wsl[:, d * 128:(d + 1) * 128],
                                     rhs=hT[d][:, bc], start=(d == 0),
                                     stop=(d == KD - 1))
                nc.scalar.activation(out=kbig[:, (ot - 6) * TB:(ot - 5) * TB],
                                     in_=pv, func=AF.Identity, bias=bq2[:, ot:ot + 1])
            kv_in = dram.tile([1, TOT2], BF16, name="kv_in", tag=f"kv_in{b}")
            kv_out = dram.tile([1, NCORES * TOT2], BF16, name="kv_out",
                               tag=f"kv_out{b}", addr_space="Shared")
            kvf = kv_in[:, :].rearrange("o n -> (o n)")
            nc.sync.dma_start(
                out=kvf[0:TOT2].rearrange("(p x) -> p x", p=128)[:, 0:KB],
                in_=kbig[:, :])
            vbig = rot.tile([128, VB], BF16, name="vbig", tag=f"vbig{b}", bufs=1)
            vbig4 = vbig[:, :].rearrange("p (t h e) -> p t h e", t=2, h=H)
            nc.gpsimd.memset(vbig4[:, :, :, 64:65], 1.0)
            for tt in range(2):
                ps = psa()
                for d in range(KD):
                    for hv in range(2):  # out must stay within one PSUM bank
                        nc.tensor.matmul(
                            out=ps[:, hv * 512:hv * 512 + 512 - hv * 256],
                            lhsT=hT[d][:, b * TB + tt * 128:b * TB + (tt + 1) * 128],
                            rhs=wv2[d][:, hv * 512:hv * 512 + 512 - hv * 256],
                            start=(d == 0), stop=(d == KD - 1))
                nc.vector.tensor_tensor(
                    out=vbig4[:, tt, :, 0:64],
                    in0=ps[:, 0:D].rearrange("p (h e) -> p h e", e=64),
                    in1=bvb[:, :].rearrange("p (h e) -> p h e", e=64),
                    op=ALU.add)
            nc.sync.dma_start(
                out=kvf[0:TOT2].rearrange("(p x) -> p x", p=128)[:, KB:KB + VB],
                in_=vbig[:, :])
            nc.gpsimd.collective_compute(
                "AllGather", ALU.bypass, replica_groups=[list(range(NCORES))],
                ins=[kv_in[:, :].opt()], outs=[kv_out[:, :].opt()])
            return kv_out

        def q_phase(lx, bq2):
            for ot in range(6):
                wsl = wslab(wqkC[lx], ot)
                ps = psa()
                pv = ps[:, 0:PT]
                for d in range(KD):
                    nc.tensor.matmul(out=pv, lhsT=wsl[:, d * 128:(d + 1) * 128],
                                     rhs=hT[d][:, :], start=(d == 0),
                                     stop=(d == KD - 1))
                nc.scalar.activation(out=qT[ot][:, :], in_=pv,
                                     func=AF.Identity, bias=bq2[:, ot:ot + 1])

        def load_kvg(kvo_h):
            kvof = kvo_h[:, :].rearrange("o n -> (o n)")
            for c in range(NCORES):
                nc.sync.dma_start(
                    out=kvg[c][:, :],
                    in_=kvof[c * TOT2:(c + 1) * TOT2].rearrange("(p x) -> p x", p=128))

        def attn_b(b):
            """Attention for batch b over all 6 head pairs and 8 source cores.
            Pair scores share one 2-bank PSUM slot (hh0 bank0, hh1 bank1);
            causal diag masks applied with one strided DVE multiply per
            (core, pair); softmax normalize batched across the pair."""
            bc = slice(b * TB, (b + 1) * TB)
            for pr in range(6):
                qtile = qT[pr]
                # one accumulator tile PER HEAD: concurrent accumulation
                # groups must not share a PSUM bank
                y_ps = [psy(), psy()]
                for c in range(NCORES):
                    S = psa()
                    Sv = [S[:, 0:512], S[:, 512:1024]]
                    for kt in range(2):
                        for hh in range(2):
                            kp = hh * 64
                            nc.tensor.matmul(
                                out=Sv[hh][:, kt * 256:kt * 256 + 256 - kt * 128],
                                lhsT=ktg[c][kp:kp + 64,
                                            pr * TB + kt * 128:pr * TB + (kt + 1) * 128],
                                rhs=qtile[kp:kp + 64, b * TB + kt * 128:(b + 1) * TB],
                                start=(kt == 0), stop=(kt == 1))
                    es = esp.tile([128, 2 * 384], BF16, name="es", tag="es")
                    nc.scalar.activation(
                        out=es[:, :].rearrange("p (h q) -> p h q", h=2),
                        in_=S[:, :].rearrange("p (h q) -> p h q", h=2)[:, :, 0:384],
                        func=AF.Exp, bias=zero_col[:, :], scale=0.125)
                    es4 = es[:, :].rearrange("p (h a q) -> p h a q", h=2, q=128)
                    m2v = m2_sb[:, c * 256:(c + 1) * 256].rearrange(
                        "p (h q) -> p h q", h=2)
                    for a in (0, 2):  # the two diagonal 128x128 blocks
                        nc.vector.tensor_tensor(
                            out=es4[:, :, a, :], in0=es4[:, :, a, :],
                            in1=m2v, op=ALU.mult)
                    es2 = es[:, :].rearrange("p (h q) -> p h q", h=2)
                    for hh in range(2):
                        hx = 2 * pr + hh
                        nc.tensor.matmul(
                            out=y_ps[hh][0:VW, 0:256],
                            lhsT=vgf[c][:, hx * VW:(hx + 1) * VW],
                            rhs=es2[:, hh, 0:256], start=(c == 0), stop=False)
                        nc.tensor.matmul(
                            out=y_ps[hh][0:VW, 128:256],
                            lhsT=vgf[c][:, VH12 + hx * VW:VH12 + (hx + 1) * VW],
                            rhs=es2[:, hh, 256:384],
                            start=False, stop=(c == NCORES - 1))
                # evacuate fast; normalize (one reciprocal per PAIR) off-path
                y_sb = rot.tile([65, PT], BF16, name="y_sb", tag="y_sb", bufs=3)
                zden = rowp.tile([1, PT], F32, name="zden", tag="row")
                for hh in range(2):
                    nc.vector.tensor_copy(out=y_sb[0:65, hh * 256:(hh + 1) * 256],
                                          in_=y_ps[hh][0:65, 0:256])
                    nc.vector.tensor_copy(out=zden[:, hh * 256:(hh + 1) * 256],
                                          in_=y_ps[hh][64:65, 0:256])
                zrec = rowp.tile([1, PT], F32, name="zrec", tag="row")
                nc.vector.reciprocal_approx_fast(out=zrec[:, :], in_=zden[:, :])
                bcb = psy()
                nc.tensor.matmul(out=bcb[0:64, 0:PT], lhsT=ones_row[:, 0:64],
                                 rhs=zrec[:, :], start=True, stop=True)
                for hh in range(2):
                    hx = 2 * pr + hh
                    nc.vector.tensor_tensor(
                        out=yT[hx // 2][(hx % 2) * 64:(hx % 2) * 64 + 64, bc],
                        in0=y_sb[0:64, hh * 256:(hh + 1) * 256],
                        in1=bcb[0:64, hh * 256:(hh + 1) * 256], op=ALU.mult)

        def out_proj_b(b):
            bc = slice(b * TB, (b + 1) * TB)
            oslot = [psa() for _ in range(3)]
            oacc = [oslot[o // 2][:, (o % 2) * 512:(o % 2) * 512 + TB]
                    for o in range(KD)]
            for k in range(KD):
                for o in range(KD):
                    nc.tensor.matmul(out=oacc[o],
                                     lhsT=w768[k][:, o * 128:(o + 1) * 128],
                                     rhs=yT[k][:, bc], start=(k == 0),
                                     stop=(k == KD - 1))
            for o in range(KD):
                nc.vector.tensor_add(out=xT[o][:, bc], in0=xT[o][:, bc],
                                     in1=oacc[o])

        def ff_b(lx, b, bft):
            bc = slice(b * TB, (b + 1) * TB)
            fslot = [psa() for _ in range(3)]
            facc = [fslot[o // 2][:, (o % 2) * 512:(o % 2) * 512 + TB]
                    for o in range(KD)]
            for ot in range(24):
                wsl = wslab(w1C[lx], ot)
                ps = psy()
                for d in range(KD):
                    nc.tensor.matmul(out=ps[:, 0:TB],
                                     lhsT=wsl[:, d * 128:(d + 1) * 128],
                                     rhs=hT[d][:, bc], start=(d == 0),
                                     stop=(d == KD - 1))
                g = gp.tile([128, TB], BF16, name="g", tag="gb")
                nc.scalar.activation(out=g[:, :], in_=ps[:, 0:TB], func=AF.Gelu,
                                     bias=bft[:, ot:ot + 1], scale=1.0)
                w2s = w2p.tile([128, D], BF16, name="w2s", tag="w2s")
                nc.sync.dma_start(out=w2s[:, :], in_=w2[lx][ot * 128:(ot + 1) * 128, :])
                for o in range(KD):
                    nc.tensor.matmul(out=facc[o], lhsT=w2s[:, o * 128:(o + 1) * 128],
                                     rhs=g[:, :], start=(ot == 0), stop=(ot == 23))
            for o in range(KD):
                nc.vector.tensor_add(out=xT[o][:, bc], in0=xT[o][:, bc],
                                     in1=facc[o])

        # ================= embedding =================
        posv = rot.tile([128, KD * PT], BF16, name="posv", tag="posv", bufs=1)
        for d in range(KD):
            nc.sync.dma_start(out=posv[:, d * PT:(d + 1) * PT],
                              in_=posT[d * 128:(d + 1) * 128, :])

        def embed_tiles(tts):
            for tt in tts:
                xg = rot.tile([128, D], F32, name="xg", tag="xg", bufs=2)
                nc.gpsimd.indirect_dma_start(
                    out=xg[:, :], out_offset=None, in_=toke[:, :],
                    in_offset=bass.IndirectOffsetOnAxis(ap=idx_sb[:, tt:tt + 1], axis=0))
                for dp in range(3):  # d-pairs share a 2-bank slot
                    tp = psa()
                    for k in range(2):
                        d = 2 * dp + k
                        sub = tp[:, k * 512:k * 512 + 128]
                        nc.tensor.transpose(out=sub, in_=xg[:, d * 128:(d + 1) * 128],
                                            identity=ident[:, :])
                        nc.vector.tensor_tensor(
                            out=xT[d][:, tt * 128:(tt + 1) * 128], in0=sub,
                            in1=posv[:, d * PT + tt * 128:d * PT + (tt + 1) * 128],
                            op=ALU.add)

        # ================= pipelined prologue + layers =================
        # Steady state per layer: attn(b0) | out/LN2/FF(b0) | LN1'+KV'+AG'(b0)
        # | attn(b1) | out/LN2/FF(b1) | LN1'+KV'+AG'(b1) | Q'. Each AllGather
        # is triggered ~100us of compute before its consumer, so the
        # collectives fly entirely under compute.
        kvo = [None, None]
        if PIPELINE:
            bq2, wv2, bvb = load_kv_weights(0)
            embed_tiles([0, 1])
            kvo[0] = kv_phase(0, 0, bq2, wv2, bvb)
            embed_tiles([2, 3])
            kvo[1] = kv_phase(0, 1, bq2, wv2, bvb)
            q_phase(0, bq2)
            load_kvg(kvo[0])
        else:
            embed_tiles([0, 1, 2, 3])

        for l in range(L):
            for k in range(KD):
                nc.sync.dma_start(out=w768[k][:, :],
                                  in_=wout[l][k * 128:(k + 1) * 128, :])
            bft = biasp.tile([128, 24], F32, name="bft", tag="bft")
            nc.sync.dma_start(out=bft[:, :], in_=b1[l][:, :])
            if not PIPELINE:
                bq2, wv2, bvb = load_kv_weights(l)
                kvo[0] = kv_phase(l, 0, bq2, wv2, bvb)
                kvo[1] = kv_phase(l, 1, bq2, wv2, bvb)
                q_phase(l, bq2)
                load_kvg(kvo[0])
            elif l + 1 < L:
                bq2, wv2, bvb = load_kv_weights(l + 1)
            # prefetch the Exp activation table before attention
            texp = rowp.tile([1, PT], F32, name="texp", tag="row")
            nc.scalar.activation(out=texp[0:1, 0:1], in_=eps_t[:, :],
                                 func=AF.Exp, bias=eps_t[:, :])
            for b in range(2):
                attn_b(b)
                if b == 0:
                    load_kvg(kvo[1])
                out_proj_b(b)
                ln_b(hT, b)          # LN2
                ff_b(l, b, bft)
                if PIPELINE and l + 1 < L:
                    kvo[b] = kv_phase(l + 1, b, bq2, wv2, bvb)
            if PIPELINE and l + 1 < L:
                q_phase(l + 1, bq2)
                load_kvg(kvo[0])

        # ================= final LN + lm_head =================
        ln_b(hT, 0)
        ln_b(hT, 1)
        for vc in range(NV2):
            esl = embp.tile([128, KD * 1024], BF16, name="esl", tag="esl")
            nc.sync.dma_start(out=esl[:, :],
                              in_=embC[:, vc * KD * 1024:(vc + 1) * KD * 1024])
            esl3 = esl[:, :].rearrange("p (d v) -> p d v", v=1024)
            for tp in range(2):
                lsb = logp.tile([128, 2 * 1024], BF16, name="lsb", tag="lsb")
                lsb3 = lsb[:, :].rearrange("p (t v) -> p t v", v=1024)
                for k in range(2):
                    tt = 2 * tp + k
                    ps = psa()
                    for d in range(KD):
                        for hv in range(2):  # matmul out must stay in one bank
                            nc.tensor.matmul(out=ps[:, hv * 512:(hv + 1) * 512],
                                             lhsT=hT[d][:, tt * 128:(tt + 1) * 128],
                                             rhs=esl3[:, d, hv * 512:(hv + 1) * 512],
                                             start=(d == 0), stop=(d == KD - 1))
                    if tt % 2 == 0:
                        nc.vector.tensor_copy(out=lsb3[:, k, :], in_=ps[:, :])
                    else:
                        nc.scalar.activation(out=lsb3[:, k, :], in_=ps[:, :],
                                             func=AF.Identity, bias=zero_col[:, :])
                nc.sync.dma_start(
                    out=logits_t[:, vc * NT * 1024 + tp * 2048:
                                 vc * NT * 1024 + (tp + 1) * 2048],
                    in_=lsb[:, :])
    nc.finalize()
    return nc


# ------------------------------------------------------------------
# host side
# ------------------------------------------------------------------

def _prep_inputs(nb, L, V, idx, tok_emb, pos_emb, ln1_w, ln1_b, qkv_w, out_w,
                 ln2_w, ln2_b, ff1_w, ff2_w, lnf_w, lnf_b):
    NT = 2 * nb
    PT = NT * 128
    idx = np.asarray(idx).astype(np.int32)
    f = np.asarray

    V_ = tok_emb.shape[0]
    NV2 = (V_ + 1023) // 1024
    embW = (f(tok_emb, dtype=np.float32) * f(lnf_w, dtype=np.float32)[None, :]).T  # [D, V]
    embP = np.zeros((D, NV2 * 1024), np.float32)
    embP[:, :V_] = embW
    embC = embP.reshape(KD, 128, NV2, 1024).transpose(1, 2, 0, 3)
    shared = {
        "toke": f(tok_emb, dtype=np.float32),
        "embC": np.ascontiguousarray(embC.reshape(128, NV2 * KD * 1024)).astype(bf16),
    }
    for l in range(L):
        wq = f(qkv_w[l], dtype=np.float32) * f(ln1_w[l], dtype=np.float32)[:, None]
        bq_full = f(ln1_b[l], dtype=np.float32) @ f(qkv_w[l], dtype=np.float32)  # [3D]
        # Q/K slabs tiled [p, ot, d*128+c] (ot 0..5 = Q, 6..11 = K)
        wqk = wq[:, :2 * D].reshape(KD, 128, 12, 128).transpose(1, 2, 0, 3)
        shared[f"wqkC{l}"] = np.ascontiguousarray(wqk.reshape(128, 12 * D)).astype(bf16)
        # V weights tiled [p, d, hk*384+e]
        wv_ = wq[:, 2 * D:].reshape(KD, 128, D).transpose(1, 0, 2)
        shared[f"wvC{l}"] = np.ascontiguousarray(wv_.reshape(128, KD * D)).astype(bf16)
        shared[f"bqkv{l}"] = np.ascontiguousarray(bq_full[:12 * 128].reshape(12, 128).T).astype(np.float32)
        shared[f"bqv{l}"] = bq_full[2 * D:].reshape(1, D).astype(np.float32)
        shared[f"wout{l}"] = f(out_w[l], dtype=np.float32).astype(bf16)
        w1e = f(ff1_w[l], dtype=np.float32) * f(ln2_w[l], dtype=np.float32)[:, None]
        b1_full = f(ln2_b[l], dtype=np.float32) @ f(ff1_w[l], dtype=np.float32)  # [4D]
        w1t = w1e.reshape(KD, 128, 24, 128).transpose(1, 2, 0, 3)
        shared[f"w1C{l}"] = np.ascontiguousarray(w1t.reshape(128, 24 * D)).astype(bf16)
        shared[f"b1_{l}"] = np.ascontiguousarray(b1_full.reshape(24, 128).T).astype(np.float32)
        shared[f"w2_{l}"] = f(ff2_w[l], dtype=np.float32).astype(bf16)

    pos_f = f(pos_emb, dtype=np.float32)
    in_maps = []
    for c in range(NCORES):
        m = dict(shared)
        L_loc = np.arange(PT)
        b_loc = L_loc // (nb * 128)
        t_loc = 8 * (L_loc % (nb * 128)) + c
        idx_core = idx[b_loc, t_loc]  # [PT]
        m["idxs"] = np.ascontiguousarray(idx_core.reshape(NT, 128).T).astype(np.int32)
        m["posT"] = np.ascontiguousarray(pos_f[t_loc].T).astype(bf16)
        # diagonal-block causal masks per source core cp, replicated x2 for the
        # two heads of a pair: keep k <= q for cp <= c, k < q for cp > c.
        mk2 = np.zeros((128, NCORES * 256), dtype=np.float32)
        for cp in range(NCORES):
            mk = np.triu(np.ones((128, 128), np.float32), 0 if cp <= c else 1)
            mk2[:, cp * 256:(cp + 1) * 256] = np.tile(mk, (1, 2))
        m["masks2"] = mk2.astype(bf16)
        in_maps.append(m)
    return in_maps


_NC_CACHE = {}


def _get_nc(nb, L, V):
    key = (nb, L, V)
    if key not in _NC_CACHE:
        _NC_CACHE[key] = build_nc(nb, L, V)
    return _NC_CACHE[key]


def run_on_hw(nb, L, V, inputs, trace=False):
    from concourse import bass_utils
    nc = _get_nc(nb, L, V)
    in_maps = _prep_inputs(nb, L, V, **inputs)
    res = bass_utils.run_bass_kernel_spmd(nc, in_maps, core_ids=list(range(NCORES)),
                                          trace=trace)
    return res


def assemble(nb, L, V, results, lnf_b, tok_emb):
    T = 8 * nb * 128
    NT = 2 * nb
    NV2 = (V + 1023) // 1024
    out = np.empty((2, T, V), dtype=np.float32)
    for c in range(NCORES):
        lt = results[c]["logits_t"].astype(np.float32).reshape(128, NV2, NT, 1024)
        lg = lt.transpose(2, 0, 1, 3).reshape(NT * 128, NV2 * 1024)[:, :V]
        out[:, c::8, :] = lg.reshape(2, nb * 128, V)
    lnf_b = np.asarray(lnf_b, dtype=np.float32)
    if np.any(lnf_b):
        out += (lnf_b @ np.asarray(tok_emb, dtype=np.float32).T)[None, None, :]
    return out


def kernel(**inputs):
    nb, L, V = 2, 6, 32000
    res = run_on_hw(nb, L, V, inputs)
    return assemble(nb, L, V, res.results, inputs["lnf_b"], inputs["tok_emb"])



# revision 58
# speedup vs baseline: 1.1391x; 1.0543x over previous
"""GPT-style dense transformer on 8 Trainium2 NeuronCores.

Sharding: token-parallel. Core c owns positions t = 8*i + c of BOTH batches
(256 positions per batch -> 512 tokens per core). All per-token work (LN,
qkv, out_proj, ff, lm_head) is local; attention needs all keys, so K^T and V
are AllGathered across the 8 cores once per layer per BATCH. The strided
assignment makes every core's causal structure identical (block-lower-
triangular over local indices, with a per-source-core diagonal mask), so one
SPMD program serves all cores.

Perf structure (v5, batch-pipelined; ~2.9ms v4 -> ~2.6ms):
- The layer is software-pipelined BY BATCH: per layer the schedule is
  attn(b0) | out/LN2/FF(b0) | LN1'+KV'(b0)+AG'(b0) | attn(b1) | ... |
  LN1'+KV'(b1)+AG'(b1) | Q'. Each 3.2MB AllGather is triggered ~100us of
  compute ahead of its consumer, so the collectives fly under compute.
  (8-rank RDH with Shared output is the fast path on this stack.)
- Attention scores per HEAD PAIR: heads (2m, 2m+1) occupy PE rows 0:64 /
  64:128 (K=64 each); adjacent issue with disjoint row groups runs both
  concurrently. The pair's scores share one 2-bank PSUM slot (hh0 bank0,
  hh1 bank1) -> ONE strided exp [128, 2, 384] per (core, pair) on ACT.
- IMPORTANT PSUM RULE (hardware): concurrent accumulation groups must live
  in DIFFERENT 2KB PSUM banks. Two interleaved start/stop groups in one bank
  corrupt each other (all-NaN + intermittent hangs). Hence y accumulators
  are one PSUM tile per head.
- Causal diag masks: multiplicative DVE ops on the exp'd scores (the es
  tiles), 2 x [128, 2, 128] per (core, pair); masks pre-tiled host-side.
- V is staged through the collective already padded with the softmax-ones
  column (65 cols/head); y and the softmax denominator accumulate in one
  PSUM pass. Normalize: ONE reciprocal_approx_fast per pair (both heads),
  broadcast via K=1 matmul into psA, final multiply on the idle GPSIMD.
- LN: stats via K=1 matmuls (1/D folded), rsqrt path uses
  reciprocal_approx_fast (5x faster than DVE reciprocal).
- Weights are STREAMED as pre-tiled [128, 768] slabs (one contiguous DMA
  each) instead of held resident: frees ~21KB/partition of SBUF.
- lm_head: embedding pre-tiled per 1024-vocab chunk [p, vc, d, v] so each
  chunk load is one contiguous-per-partition DMA; logits written in a
  core-tiled layout and untangled on the host. PE-bound at ~99% occupancy.
"""

import sys

for _p in ("/opt/trn_rl_repo",):
    if _p not in sys.path:
        sys.path.insert(0, _p)

import numpy as np
import ml_dtypes

import concourse.bass as bass
import concourse.bacc as bacc
import concourse.mybir as mybir
import concourse.tile as tile
from concourse.masks import make_identity

BF16 = mybir.dt.bfloat16
FP8 = mybir.dt.float8e4
F32 = mybir.dt.float32
I32 = mybir.dt.int32
AF = mybir.ActivationFunctionType
ALU = mybir.AluOpType

NCORES = 8
H = 12          # heads
HD = 64         # head dim
D = 768
D3 = 3 * D      # 2304
DF = 4 * D      # 3072
KD = D // 128   # 6 d-tiles
EPS = 1e-5

bf16 = ml_dtypes.bfloat16
WARM_AG = False
PIPELINE = True


def build_nc(nb, L, V, stop_at=None):
    """Build the SPMD Bass module. nb = 128-token tiles per (core, batch).
    Full size: nb=2 -> 512 tokens/core, T = 8*128*nb = 2048."""
    assert nb == 2, "v4 kernel is specialized to nb=2 (512 tokens/core)"
    NT = 2 * nb          # token tiles per core (4)
    PT = NT * 128        # tokens per core (512)
    TB = nb * 128        # tokens per batch per core (256)
    VW = 65              # V cols per head incl. ones column
    VH12 = H * VW        # V cols per key-token tile (all 12 heads, 780)
    KB = 6 * TB          # K^T staging cols per batch (6 head-pairs x 256)
    VB = 2 * VH12        # V staging cols per batch (2 key tiles x 780)
    TOT2 = 128 * (KB + VB)       # kv elems per rank per batch AllGather

    nc = bacc.Bacc("TRN2", target_bir_lowering=False, num_devices=NCORES)

    # ---- I/O ----
    NV2 = (V + 1023) // 1024          # 1024-wide vocab chunks for lm_head
    idxs = nc.dram_tensor("idxs", [128, NT], I32, kind="ExternalInput")
    posT = nc.dram_tensor("posT", [D, PT], BF16, kind="ExternalInput")
    # per-src-core causal mask for the two diagonal 128x128 blocks, replicated
    # for the two heads of a pair (h=2, q=128): two 3-D DVE multiplies per
    # (core, pair)
    masks2 = nc.dram_tensor("masks2", [128, NCORES * 256], BF16, kind="ExternalInput")
    toke = nc.dram_tensor("toke", [V, D], F32, kind="ExternalInput")
    # lm_head embedding pre-tiled per 1024-vocab chunk: [p, vc, d, v] layout so
    # each chunk load is one contiguous-per-partition DMA (128 descriptors)
    embC = nc.dram_tensor("embC", [128, NV2 * KD * 1024], BF16, kind="ExternalInput")
    # Q/K weights pre-tiled per output slab: [p, ot, d*128+c] so each slab is
    # one contiguous-per-partition DMA; V weights pre-tiled per d-tile.
    wqkC = [nc.dram_tensor(f"wqkC{l}", [128, 12 * D], BF16, kind="ExternalInput") for l in range(L)]
    wvC = [nc.dram_tensor(f"wvC{l}", [128, KD * D], BF16, kind="ExternalInput") for l in range(L)]
    bqkv = [nc.dram_tensor(f"bqkv{l}", [128, 12], F32, kind="ExternalInput") for l in range(L)]
    bqv = [nc.dram_tensor(f"bqv{l}", [1, D], F32, kind="ExternalInput") for l in range(L)]
    wout = [nc.dram_tensor(f"wout{l}", [D, D], BF16, kind="ExternalInput") for l in range(L)]
    w1C = [nc.dram_tensor(f"w1C{l}", [128, 24 * D], BF16, kind="ExternalInput") for l in range(L)]
    b1 = [nc.dram_tensor(f"b1_{l}", [128, 24], F32, kind="ExternalInput") for l in range(L)]
    w2 = [nc.dram_tensor(f"w2_{l}", [DF, D], BF16, kind="ExternalInput") for l in range(L)]
    # logits in core-tiled layout [p, vc, t, v]; host untangles (cheap)
    logits_t = nc.dram_tensor("logits_t", [128, NV2 * NT * 1024], BF16,
                              kind="ExternalOutput")

    from contextlib import ExitStack
    with tile.TileContext(nc) as tc, ExitStack() as ctx:
        def pool(**kw):
            return ctx.enter_context(tc.tile_pool(**kw))
        # ---- pools ----
        const = pool(name="const", bufs=1)
        resid = pool(name="resid", bufs=1)
        acts = pool(name="acts", bufs=1)
        kvres = pool(name="kvres", bufs=1)
        wpool = pool(name="wpool", bufs=1)
        wopool = pool(name="wopool", bufs=1)
        biasp = pool(name="biasp", bufs=2)
        rot = pool(name="rot", bufs=2)
        esp = pool(name="esp", bufs=4)
        gp = pool(name="gp", bufs=2)
        w2p = pool(name="w2p", bufs=2)
        embp = pool(name="embp", bufs=2)
        logp = pool(name="logp", bufs=2)
        rowp = pool(name="rowp", bufs=3)
        psA = pool(name="psA", bufs=3, space="PSUM")   # [128,1024] 2-bank slots
        psY = pool(name="psY", bufs=2, space="PSUM")   # [128,512] 1-bank slots
        dram = pool(name="dram", bufs=2, space="DRAM")

        def psa():
            return psA.tile([128, 1024], F32, name="sa", tag="s2")

        def psy():
            return psY.tile([128, PT], F32, name="sy", tag="y")

        # ---- constants ----
        ident = const.tile([128, 128], F32, name="ident", tag="ident")
        make_identity(nc, ident)
        ones_col = const.tile([128, 1], BF16, name="ones_col", tag="ones_col")
        nc.gpsimd.memset(ones_col[:, :], 1.0)
        invd_col = const.tile([128, 1], BF16, name="invd_col", tag="invd_col")
        nc.gpsimd.memset(invd_col[:, :], 1.0 / D)
        ones_row = const.tile([1, 128], F32, name="ones_row", tag="ones_row")
        nc.gpsimd.memset(ones_row[:, :], 1.0)
        eps_t = const.tile([1, 1], F32, name="eps_t", tag="eps_t")
        nc.gpsimd.memset(eps_t[:, :], EPS)
        zero_col = const.tile([128, 1], F32, name="zero_col", tag="zero_col")
        nc.gpsimd.memset(zero_col[:, :], 0.0)
        m2_sb = const.tile([128, NCORES * 256], BF16, name="m2_sb", tag="m2_sb")
        nc.sync.dma_start(out=m2_sb[:, :], in_=masks2[:, :])
        idx_sb = const.tile([128, NT], I32, name="idx_sb", tag="idx_sb")
        nc.sync.dma_start(out=idx_sb[:, :], in_=idxs[:, :])

        # tiny warm-up AllGather: wakes the collectives firmware during the
        # embedding phase so layer 0's first real AllGather doesn't pay the
        # ~100us first-collective latency observed in traces.
        if WARM_AG:
            warm_in = dram.tile([1, 256], BF16, name="warm_in", tag="warm_in")
            warm_out = dram.tile([1, NCORES * 256], BF16, name="warm_out",
                                 tag="warm_out", addr_space="Shared")
            nc.sync.dma_start(out=warm_in[:, :], in_=posT[0:1, 0:256])
            nc.gpsimd.collective_compute(
                "AllGather", ALU.bypass, replica_groups=[list(range(NCORES))],
                ins=[warm_in[:, :].opt()], outs=[warm_out[:, :].opt()])
            warm_sb = const.tile([1, 256], BF16, name="warm_sb", tag="warm_sb")
            nc.sync.dma_start(out=warm_sb[:, :], in_=warm_out[:, 0:256])

        # ---- persistent per-layer state ----
        xT = [resid.tile([128, PT], F32, name=f"xt{d}", tag=f"xt{d}") for d in range(KD)]
        hT = [acts.tile([128, PT], BF16, name=f"ht{d}", tag=f"ht{d}") for d in range(KD)]
        qT = [acts.tile([128, PT], BF16, name=f"qt{d}", tag=f"qt{d}") for d in range(KD)]
        yT = [acts.tile([128, PT], BF16, name=f"yt{d}", tag=f"yt{d}") for d in range(KD)]
        # gathered K^T + V per source core (ONE batch at a time), fused in ONE
        # tile so each (core, batch) load is a single contiguous-per-partition
        # DMA. ktg view: [128 pair-rows, 6 pairs, 256 tok]; vgf view:
        # [128 tok, 2 key tiles, 12 heads, 65].
        kvg = [kvres.tile([128, KB + VB], BF16, name=f"kvg{c}",
                          tag=f"kvg{c}") for c in range(NCORES)]
        ktg = [t[:, 0:KB] for t in kvg]
        vgf = [t[:, KB:KB + VB] for t in kvg]

        w768 = [wopool.tile([128, D], BF16, name=f"w7{i}", tag=f"w7{i}") for i in range(KD)]

        def wslab(src, ot):
            """Stream one [128, 6*128] weight slab (all 6 d-tiles of output
            slab ot) from a pre-tiled DRAM layout; one contiguous DMA."""
            t = wpool.tile([128, D], BF16, name="wsl", tag="wsl", bufs=4)
            nc.sync.dma_start(out=t[:, :], in_=src[:, ot * D:(ot + 1) * D])
            return t

        def ln_b(dst_bf16, _b):
            """dst[d][:, b*TB:(b+1)*TB] <- layernorm of batch-b cols of xT."""
            b = _b
            bc = slice(b * TB, (b + 1) * TB)
            s12 = psa()  # bank0: sum, bank1: sum of squares
            s1 = s12[0:1, 0:TB]
            s2 = s12[0:1, 512:512 + TB]
            for d in range(KD):
                xb = rot.tile([128, TB], BF16, name="xb", tag="xbb")
                nc.vector.tensor_copy(out=xb[:, :], in_=xT[d][:, bc])
                sq = rot.tile([128, TB], BF16, name="sq", tag="sqb")
                # square on the idle GPSIMD: halves the DVE chain feeding
                # the LN stats matmuls
                nc.gpsimd.tensor_tensor(out=sq[:, :], in0=xb[:, :],
                                        in1=xb[:, :], op=ALU.mult)
                nc.tensor.matmul(out=s1, lhsT=invd_col[:, :], rhs=xb[:, :],
                                 start=(d == 0), stop=(d == KD - 1))
                nc.tensor.matmul(out=s2, lhsT=invd_col[:, :], rhs=sq[:, :],
                                 start=(d == 0), stop=(d == KD - 1))
            # s1 = E[x], s2 = E[x^2] directly (1/D folded into the matmul
            # constant); square on ACT to avoid a same-bank double PSUM read
            msq = rowp.tile([1, TB], F32, name="msq", tag="rowb")
            nc.scalar.activation(out=msq[:, :], in_=s1, func=AF.Square,
                                 bias=zero_col[0:1, :])
            vrow = rowp.tile([1, TB], F32, name="vrow", tag="rowb")
            nc.vector.tensor_tensor(out=vrow[:, :], in0=s2, in1=msq[:, :],
                                    op=ALU.subtract)
            srow = rowp.tile([1, TB], F32, name="srow", tag="rowb")
            nc.scalar.activation(out=srow[:, :], in_=vrow[:, :], func=AF.Sqrt,
                                 bias=eps_t[:, :])
            rrow = rowp.tile([1, TB], F32, name="rrow", tag="rowb")
            nc.vector.reciprocal_approx_fast(out=rrow[:, :], in_=srow[:, :])
            mr = rowp.tile([1, TB], F32, name="mr", tag="rowb")
            nc.vector.tensor_mul(out=mr[:, :], in0=s1, in1=rrow[:, :])
            # broadcast [1, TB] rows across 128 partitions via K=1 matmul
            bcpair = psa()
            bc_r = bcpair[:, 0:TB]
            bc_mr = bcpair[:, 512:512 + TB]
            nc.tensor.matmul(out=bc_r, lhsT=ones_row[:, :], rhs=rrow[:, :],
                             start=True, stop=True)
            nc.tensor.matmul(out=bc_mr, lhsT=ones_row[:, :], rhs=mr[:, :],
                             start=True, stop=True)
            for d in range(KD):
                t32 = rot.tile([128, TB], BF16, name="t32", tag="t32b")
                nc.vector.tensor_mul(out=t32[:, :], in0=xT[d][:, bc], in1=bc_r)
                nc.vector.tensor_sub(out=dst_bf16[d][:, bc], in0=t32[:, :], in1=bc_mr)

        # ---- per-layer phase helpers (batch-pipelined schedule) ----
        def load_kv_weights(lx):
            """Biases + V weights for layer lx's K/V projections, plus the
            V-bias broadcast to all 128 partitions."""
            bq2 = biasp.tile([128, 12], F32, name="bq", tag="bq")
            nc.sync.dma_start(out=bq2[:, :], in_=bqkv[lx][:, :])
            bv2 = biasp.tile([1, D], F32, name="bv", tag="bv")
            nc.sync.dma_start(out=bv2[:, :], in_=bqv[lx][:, :])
            wv2 = [wpool.tile([128, D], BF16, name=f"wv{d}", tag=f"wv{d}")
                   for d in range(KD)]
            for d in range(KD):
                nc.sync.dma_start(out=wv2[d][:, :], in_=wvC[lx][:, d * D:(d + 1) * D])
            bvb = rot.tile([128, D], F32, name="bvb", tag="bvb", bufs=2)
            for vh in range(2):
                bcv = psy()
                nc.tensor.matmul(out=bcv[:, 0:384], lhsT=ones_row[:, :],
                                 rhs=bv2[:, vh * 384:(vh + 1) * 384],
                                 start=True, stop=True)
                nc.vector.tensor_copy(out=bvb[:, vh * 384:(vh + 1) * 384],
                                      in_=bcv[:, 0:384])
            return bq2, wv2, bvb

        def kv_phase(lx, b, bq2, wv2, bvb):
            """LN1(lx, b) -> K^T/V projections for batch b (all 12 heads) ->
            stage to DRAM -> trigger the batch-b AllGather."""
            bc = slice(b * TB, (b + 1) * TB)
            ln_b(hT, b)
            kbig = rot.tile([128, KB], BF16, name="kbig", tag=f"kbig{b}", bufs=1)
            for ot in range(6, 12):
                wsl = wslab(wqkC[lx], ot)
                ps = psa()
                pv = ps[:, 0:TB]
                for d in range(KD):
                    nc.tensor.matmul(out=pv, lhsT=wsl[:, d * 128:(d + 1) * 128],
                                     rhs=hT[d][:, bc], start=(d == 0),
                                     stop=(d == KD - 1))
                nc.scalar.activation(out=kbig[:, (ot - 6) * TB:(ot - 5) * TB],
                                     in_=pv, func=AF.Identity, bias=bq2[:, ot:ot + 1])
            kv_in = dram.tile([1, TOT2], BF16, name="kv_in", tag=f"kv_in{b}")
            kv_out = dram.tile([1, NCORES * TOT2], BF16, name="kv_out",
                               tag=f"kv_out{b}", addr_space="Shared")
            kvf = kv_in[:, :].rearrange("o n -> (o n)")
            nc.sync.dma_start(
                out=kvf[0:TOT2].rearrange("(p x) -> p x", p=128)[:, 0:KB],
                in_=kbig[:, :])
            vbig = rot.tile([128, VB], BF16, name="vbig", tag=f"vbig{b}", bufs=1)
            vbig4 = vbig[:, :].rearrange("p (t h e) -> p t h e", t=2, h=H)
            nc.gpsimd.memset(vbig4[:, :, :, 64:65], 1.0)
            for tt in range(2):
                ps = psa()
                for d in range(KD):
                    for hv in range(2):  # out must stay within one PSUM bank
                        nc.tensor.matmul(
                            out=ps[:, hv * 512:hv * 512 + 512 - hv * 256],
                            lhsT=hT[d][:, b * TB + tt * 128:b * TB + (tt + 1) * 128],
                            rhs=wv2[d][:, hv * 512:hv * 512 + 512 - hv * 256],
                            start=(d == 0), stop=(d == KD - 1))
                nc.vector.tensor_tensor(
                    out=vbig4[:, tt, :, 0:64],
                    in0=ps[:, 0:D].rearrange("p (h e) -> p h e", e=64),
                    in1=bvb[:, :].rearrange("p (h e) -> p h e", e=64),
                    op=ALU.add)
            nc.sync.dma_start(
                out=kvf[0:TOT2].rearrange("(p x) -> p x", p=128)[:, KB:KB + VB],
                in_=vbig[:, :])
            nc.gpsimd.collective_compute(
                "AllGather", ALU.bypass, replica_groups=[list(range(NCORES))],
                ins=[kv_in[:, :].opt()], outs=[kv_out[:, :].opt()])
            return kv_out

        def q_phase(lx, bq2):
            for ot in range(6):
                wsl = wslab(wqkC[lx], ot)
                ps = psa()
                pv = ps[:, 0:PT]
                for d in range(KD):
                    nc.tensor.matmul(out=pv, lhsT=wsl[:, d * 128:(d + 1) * 128],
                                     rhs=hT[d][:, :], start=(d == 0),
                                     stop=(d == KD - 1))
                nc.scalar.activation(out=qT[ot][:, :], in_=pv,
                                     func=AF.Identity, bias=bq2[:, ot:ot + 1])

        def load_kvg(kvo_h):
            kvof = kvo_h[:, :].rearrange("o n -> (o n)")
            for c in range(NCORES):
                nc.sync.dma_start(
                    out=kvg[c][:, :],
                    in_=kvof[c * TOT2:(c + 1) * TOT2].rearrange("(p x) -> p x", p=128))

        def attn_b(b):
            """Attention for batch b over all 6 head pairs and 8 source cores.
            Pair scores share one 2-bank PSUM slot (hh0 bank0, hh1 bank1);
            causal diag masks applied with one strided DVE multiply per
            (core, pair); softmax normalize batched across the pair."""
            bc = slice(b * TB, (b + 1) * TB)
            for pr in range(6):
                qtile = qT[pr]
                # one accumulator tile PER HEAD: concurrent accumulation
                # groups must not share a PSUM bank
                y_ps = [psy(), psy()]
                for c in range(NCORES):
                    S = psa()
                    Sv = [S[:, 0:512], S[:, 512:1024]]
                    for kt in range(2):
                        for hh in range(2):
                            kp = hh * 64
                            nc.tensor.matmul(
                                out=Sv[hh][:, kt * 256:kt * 256 + 256 - kt * 128],
                                lhsT=ktg[c][kp:kp + 64,
                                            pr * TB + kt * 128:pr * TB + (kt + 1) * 128],
                                rhs=qtile[kp:kp + 64, b * TB + kt * 128:(b + 1) * TB],
                                start=(kt == 0), stop=(kt == 1))
                    es = esp.tile([128, 2 * 384], BF16, name="es", tag="es")
                    nc.scalar.activation(
                        out=es[:, :].rearrange("p (h q) -> p h q", h=2),
                        in_=S[:, :].rearrange("p (h q) -> p h q", h=2)[:, :, 0:384],
                        func=AF.Exp, bias=zero_col[:, :], scale=0.125)
                    es4 = es[:, :].rearrange("p (h a q) -> p h a q", h=2, q=128)
                    m2v = m2_sb[:, c * 256:(c + 1) * 256].rearrange(
                        "p (h q) -> p h q", h=2)
                    for a in (0, 2):  # the two diagonal 128x128 blocks
                        nc.vector.tensor_tensor(
                            out=es4[:, :, a, :], in0=es4[:, :, a, :],
                            in1=m2v, op=ALU.mult)
                    es2 = es[:, :].rearrange("p (h q) -> p h q", h=2)
                    for hh in range(2):
                        hx = 2 * pr + hh
                        nc.tensor.matmul(
                            out=y_ps[hh][0:VW, 0:256],
                            lhsT=vgf[c][:, hx * VW:(hx + 1) * VW],
                            rhs=es2[:, hh, 0:256], start=(c == 0), stop=False)
                        nc.tensor.matmul(
                            out=y_ps[hh][0:VW, 128:256],
                            lhsT=vgf[c][:, VH12 + hx * VW:VH12 + (hx + 1) * VW],
                            rhs=es2[:, hh, 256:384],
                            start=False, stop=(c == NCORES - 1))
                # evacuate fast; normalize (one reciprocal per PAIR) off-path
                y_sb = rot.tile([65, PT], BF16, name="y_sb", tag="y_sb", bufs=3)
                zden = rowp.tile([1, PT], F32, name="zden", tag="row")
                for hh in range(2):
                    nc.vector.tensor_copy(out=y_sb[0:65, hh * 256:(hh + 1) * 256],
                                          in_=y_ps[hh][0:65, 0:256])
                    nc.vector.tensor_copy(out=zden[:, hh * 256:(hh + 1) * 256],
                                          in_=y_ps[hh][64:65, 0:256])
                zrec = rowp.tile([1, PT], F32, name="zrec", tag="row")
                nc.vector.reciprocal_approx_fast(out=zrec[:, :], in_=zden[:, :])
                bcb = psy()
                nc.tensor.matmul(out=bcb[0:64, 0:PT], lhsT=ones_row[:, 0:64],
                                 rhs=zrec[:, :], start=True, stop=True)
                for hh in range(2):
                    hx = 2 * pr + hh
                    nc.vector.tensor_tensor(
                        out=yT[hx // 2][(hx % 2) * 64:(hx % 2) * 64 + 64, bc],
                        in0=y_sb[0:64, hh * 256:(hh + 1) * 256],
                        in1=bcb[0:64, hh * 256:(hh + 1) * 256], op=ALU.mult)

        def out_proj_b(b):
            bc = slice(b * TB, (b + 1) * TB)
            oslot = [psa() for _ in range(3)]
            oacc = [oslot[o // 2][:, (o % 2) * 512:(o % 2) * 512 + TB]
                    for o in range(KD)]
            for k in range(KD):
                for o in range(KD):
                    nc.tensor.matmul(out=oacc[o],
                                     lhsT=w768[k][:, o * 128:(o + 1) * 128],
                                     rhs=yT[k][:, bc], start=(k == 0),
                                     stop=(k == KD - 1))
            for o in range(KD):
                nc.vector.tensor_add(out=xT[o][:, bc], in0=xT[o][:, bc],
                                     in1=oacc[o])

        def ff_b(lx, b, bft):
            bc = slice(b * TB, (b + 1) * TB)
            fslot = [psa() for _ in range(3)]
            facc = [fslot[o // 2][:, (o % 2) * 512:(o % 2) * 512 + TB]
                    for o in range(KD)]
            for ot in range(24):
                wsl = wslab(w1C[lx], ot)
                ps = psy()
                for d in range(KD):
                    nc.tensor.matmul(out=ps[:, 0:TB],
                                     lhsT=wsl[:, d * 128:(d + 1) * 128],
                                     rhs=hT[d][:, bc], start=(d == 0),
                                     stop=(d == KD - 1))
                g = gp.tile([128, TB], BF16, name="g", tag="gb")
                nc.scalar.activation(out=g[:, :], in_=ps[:, 0:TB], func=AF.Gelu,
                                     bias=bft[:, ot:ot + 1], scale=1.0)
                w2s = w2p.tile([128, D], BF16, name="w2s", tag="w2s")
                # issue weight prefetches from the near-idle GPSIMD queue so
                # the KV staging DMAs (which gate the AllGather triggers)
                # aren't queued behind ~9MB of slab traffic on the sync queue
                nc.gpsimd.dma_start(out=w2s[:, :],
                                    in_=w2[lx][ot * 128:(ot + 1) * 128, :])
                for o in range(KD):
                    nc.tensor.matmul(out=facc[o], lhsT=w2s[:, o * 128:(o + 1) * 128],
                                     rhs=g[:, :], start=(ot == 0), stop=(ot == 23))
            for o in range(KD):
                nc.vector.tensor_add(out=xT[o][:, bc], in0=xT[o][:, bc],
                                     in1=facc[o])

        # ================= embedding =================
        posv = rot.tile([128, KD * PT], BF16, name="posv", tag="posv", bufs=1)
        for d in range(KD):
            nc.sync.dma_start(out=posv[:, d * PT:(d + 1) * PT],
                              in_=posT[d * 128:(d + 1) * 128, :])

        def embed_tiles(tts):
            for tt in tts:
                xg = rot.tile([128, D], F32, name="xg", tag="xg", bufs=2)
                nc.gpsimd.indirect_dma_start(
                    out=xg[:, :], out_offset=None, in_=toke[:, :],
                    in_offset=bass.IndirectOffsetOnAxis(ap=idx_sb[:, tt:tt + 1], axis=0))
                for dp in range(3):  # d-pairs share a 2-bank slot
                    tp = psa()
                    for k in range(2):
                        d = 2 * dp + k
                        sub = tp[:, k * 512:k * 512 + 128]
                        nc.tensor.transpose(out=sub, in_=xg[:, d * 128:(d + 1) * 128],
                                            identity=ident[:, :])
                        nc.vector.tensor_tensor(
                            out=xT[d][:, tt * 128:(tt + 1) * 128], in0=sub,
                            in1=posv[:, d * PT + tt * 128:d * PT + (tt + 1) * 128],
                            op=ALU.add)

        # ================= pipelined prologue + layers =================
        # Steady state per layer: attn(b0) | out/LN2/FF(b0) | LN1'+KV'+AG'(b0)
        # | attn(b1) | out/LN2/FF(b1) | LN1'+KV'+AG'(b1) | Q'. Each AllGather
        # is triggered ~100us of compute before its consumer, so the
        # collectives fly entirely under compute.
        kvo = [None, None]
        if PIPELINE:
            bq2, wv2, bvb = load_kv_weights(0)
            embed_tiles([0, 1])
            kvo[0] = kv_phase(0, 0, bq2, wv2, bvb)
            embed_tiles([2, 3])
            kvo[1] = kv_phase(0, 1, bq2, wv2, bvb)
            q_phase(0, bq2)
            load_kvg(kvo[0])
        else:
            embed_tiles([0, 1, 2, 3])

        for l in range(L):
            for k in range(KD):
                nc.sync.dma_start(out=w768[k][:, :],
                                  in_=wout[l][k * 128:(k + 1) * 128, :])
            bft = biasp.tile([128, 24], F32, name="bft", tag="bft")
            nc.sync.dma_start(out=bft[:, :], in_=b1[l][:, :])
            if not PIPELINE:
                bq2, wv2, bvb = load_kv_weights(l)
                kvo[0] = kv_phase(l, 0, bq2, wv2, bvb)
                kvo[1] = kv_phase(l, 1, bq2, wv2, bvb)
                q_phase(l, bq2)
                load_kvg(kvo[0])
            elif l + 1 < L:
                bq2, wv2, bvb = load_kv_weights(l + 1)
            # prefetch the Exp activation table before attention
            texp = rowp.tile([1, PT], F32, name="texp", tag="row")
            nc.scalar.activation(out=texp[0:1, 0:1], in_=eps_t[:, :],
                                 func=AF.Exp, bias=eps_t[:, :])
            for b in range(2):
                attn_b(b)
                if b == 0:
                    load_kvg(kvo[1])
                out_proj_b(b)
                ln_b(hT, b)          # LN2
                ff_b(l, b, bft)
                if PIPELINE and l + 1 < L:
                    kvo[b] = kv_phase(l + 1, b, bq2, wv2, bvb)
            if PIPELINE and l + 1 < L:
                q_phase(l + 1, bq2)
                load_kvg(kvo[0])

        # ================= final LN + lm_head =================
        ln_b(hT, 0)
        ln_b(hT, 1)
        for vc in range(NV2):
            esl = embp.tile([128, KD * 1024], BF16, name="esl", tag="esl")
            nc.sync.dma_start(out=esl[:, :],
                              in_=embC[:, vc * KD * 1024:(vc + 1) * KD * 1024])
            esl3 = esl[:, :].rearrange("p (d v) -> p d v", v=1024)
            for tp in range(2):
                lsb = logp.tile([128, 2 * 1024], BF16, name="lsb", tag="lsb")
                lsb3 = lsb[:, :].rearrange("p (t v) -> p t v", v=1024)
                for k in range(2):
                    tt = 2 * tp + k
                    ps = psa()
                    for d in range(KD):
                        for hv in range(2):  # matmul out must stay in one bank
                            nc.tensor.matmul(out=ps[:, hv * 512:(hv + 1) * 512],
                                             lhsT=hT[d][:, tt * 128:(tt + 1) * 128],
                                             rhs=esl3[:, d, hv * 512:(hv + 1) * 512],
                                             start=(d == 0), stop=(d == KD - 1))
                    if tt % 2 == 0:
                        nc.vector.tensor_copy(out=lsb3[:, k, :], in_=ps[:, :])
                    else:
                        nc.scalar.activation(out=lsb3[:, k, :], in_=ps[:, :],
                                             func=AF.Identity, bias=zero_col[:, :])
                nc.sync.dma_start(
                    out=logits_t[:, vc * NT * 1024 + tp * 2048:
                                 vc * NT * 1024 + (tp + 1) * 2048],
                    in_=lsb[:, :])
    nc.finalize()
    return nc


# ------------------------------------------------------------------
# host side
# ------------------------------------------------------------------

def _prep_inputs(nb, L, V, idx, tok_emb, pos_emb, ln1_w, ln1_b, qkv_w, out_w,
                 ln2_w, ln2_b, ff1_w, ff2_w, lnf_w, lnf_b):
    NT = 2 * nb
    PT = NT * 128
    idx = np.asarray(idx).astype(np.int32)
    f = np.asarray

    V_ = tok_emb.shape[0]
    NV2 = (V_ + 1023) // 1024
    embW = (f(tok_emb, dtype=np.float32) * f(lnf_w, dtype=np.float32)[None, :]).T  # [D, V]
    embP = np.zeros((D, NV2 * 1024), np.float32)
    embP[:, :V_] = embW
    embC = embP.reshape(KD, 128, NV2, 1024).transpose(1, 2, 0, 3)
    shared = {
        "toke": f(tok_emb, dtype=np.float32),
        "embC": np.ascontiguousarray(embC.reshape(128, NV2 * KD * 1024)).astype(bf16),
    }
    for l in range(L):
        wq = f(qkv_w[l], dtype=np.float32) * f(ln1_w[l], dtype=np.float32)[:, None]
        bq_full = f(ln1_b[l], dtype=np.float32) @ f(qkv_w[l], dtype=np.float32)  # [3D]
        # Q/K slabs tiled [p, ot, d*128+c] (ot 0..5 = Q, 6..11 = K)
        wqk = wq[:, :2 * D].reshape(KD, 128, 12, 128).transpose(1, 2, 0, 3)
        shared[f"wqkC{l}"] = np.ascontiguousarray(wqk.reshape(128, 12 * D)).astype(bf16)
        # V weights tiled [p, d, hk*384+e]
        wv_ = wq[:, 2 * D:].reshape(KD, 128, D).transpose(1, 0, 2)
        shared[f"wvC{l}"] = np.ascontiguousarray(wv_.reshape(128, KD * D)).astype(bf16)
        shared[f"bqkv{l}"] = np.ascontiguousarray(bq_full[:12 * 128].reshape(12, 128).T).astype(np.float32)
        shared[f"bqv{l}"] = bq_full[2 * D:].reshape(1, D).astype(np.float32)
        shared[f"wout{l}"] = f(out_w[l], dtype=np.float32).astype(bf16)
        w1e = f(ff1_w[l], dtype=np.float32) * f(ln2_w[l], dtype=np.float32)[:, None]
        b1_full = f(ln2_b[l], dtype=np.float32) @ f(ff1_w[l], dtype=np.float32)  # [4D]
        w1t = w1e.reshape(KD, 128, 24, 128).transpose(1, 2, 0, 3)
        shared[f"w1C{l}"] = np.ascontiguousarray(w1t.reshape(128, 24 * D)).astype(bf16)
        shared[f"b1_{l}"] = np.ascontiguousarray(b1_full.reshape(24, 128).T).astype(np.float32)
        shared[f"w2_{l}"] = f(ff2_w[l], dtype=np.float32).astype(bf16)

    pos_f = f(pos_emb, dtype=np.float32)
    in_maps = []
    for c in range(NCORES):
        m = dict(shared)
        L_loc = np.arange(PT)
        b_loc = L_loc // (nb * 128)
        t_loc = 8 * (L_loc % (nb * 128)) + c
        idx_core = idx[b_loc, t_loc]  # [PT]
        m["idxs"] = np.ascontiguousarray(idx_core.reshape(NT, 128).T).astype(np.int32)
        m["posT"] = np.ascontiguousarray(pos_f[t_loc].T).astype(bf16)
        # diagonal-block causal masks per source core cp, replicated x2 for the
        # two heads of a pair: keep k <= q for cp <= c, k < q for cp > c.
        mk2 = np.zeros((128, NCORES * 256), dtype=np.float32)
        for cp in range(NCORES):
            mk = np.triu(np.ones((128, 128), np.float32), 0 if cp <= c else 1)
            mk2[:, cp * 256:(cp + 1) * 256] = np.tile(mk, (1, 2))
        m["masks2"] = mk2.astype(bf16)
        in_maps.append(m)
    return in_maps


_NC_CACHE = {}


def _get_nc(nb, L, V):
    key = (nb, L, V)
    if key not in _NC_CACHE:
        _NC_CACHE[key] = build_nc(nb, L, V)
    return _NC_CACHE[key]


def run_on_hw(nb, L, V, inputs, trace=False):
    from concourse import bass_utils
    nc = _get_nc(nb, L, V)
    in_maps = _prep_inputs(nb, L, V, **inputs)
    res = bass_utils.run_bass_kernel_spmd(nc, in_maps, core_ids=list(range(NCORES)),
                                          trace=trace)
    return res


def assemble(nb, L, V, results, lnf_b, tok_emb):
    T = 8 * nb * 128
    NT = 2 * nb
    NV2 = (V + 1023) // 1024
    out = np.empty((2, T, V), dtype=np.float32)
    for c in range(NCORES):
        lt = results[c]["logits_t"].astype(np.float32).reshape(128, NV2, NT, 1024)
        lg = lt.transpose(2, 0, 1, 3).reshape(NT * 128, NV2 * 1024)[:, :V]
        out[:, c::8, :] = lg.reshape(2, nb * 128, V)
    lnf_b = np.asarray(lnf_b, dtype=np.float32)
    if np.any(lnf_b):
        out += (lnf_b @ np.asarray(tok_emb, dtype=np.float32).T)[None, None, :]
    return out


def kernel(**inputs):
    nb, L, V = 2, 6, 32000
    res = run_on_hw(nb, L, V, inputs)
    return assemble(nb, L, V, res.results, inputs["lnf_b"], inputs["tok_emb"])

